# revision 11
# baseline (speedup 1.0000x reference)
"""MoE feed-forward (top-2 routing + shared expert) on 8 Trainium2 cores.

Strategy (expert parallel):
  - Host computes the router (tiny [T,D]@[D,E] matmul), top-2 expert ids and
    renormalized gates, then dispatches each expert's tokens (transposed,
    capacity-padded) to the core that owns that expert's weights.
  - Core e computes  ye = (silu(xe@w1_e) * (xe@w3_e)) @ w2_e, row-scaled by the
    gate, plus a 1/8 token-slice of the always-active shared expert.
  - Host scatter-adds routed outputs into the shared-expert output.

On-device matmuls use float32r (fp32 data with fp22 multiplies, fp32
accumulation) which runs the PE at full rate for free dims >= 256.

Pipeline shape: tokens are processed in chunks of <=512 columns. Phase 1
(h1/h3/swiglu-gate, weight-streaming, DMA-heavy) of chunk i+1 is interleaved
with phase 2 (down-projection, DMA-light) of chunk i via a pending-unit queue,
smoothing DMA demand and keeping the PE stream dense.
"""

import numpy as np

import concourse.bass as bass
import concourse.mybir as mybir
import concourse.tile as tile
from concourse import bacc
from concourse.bass_utils import run_bass_kernel_spmd

P = 128
N_CORES = 8
F32 = mybir.dt.float32
F32R = mybir.dt.float32r
AF = mybir.ActivationFunctionType

# h-tiles of w1/w3 fetched per DMA (bigger transfers, fewer descriptors)
H_BLOCK = 2


def _chunk_widths(n):
    """Split n (multiple of 128) into widths of 256..512 (multiples of 128):
    float32r matmuls run at full PE rate only for free dim >= 256.
    Smallest chunk first for a faster pipeline ramp."""
    assert n % P == 0
    if n < 2 * P:
        return [n]
    widths = []
    while n > 0:
        if n >= 640 or n == 512:
            widths.append(512)
            n -= 512
        else:  # 256, 384 (n==640 handled above)
            widths.append(n)
            n = 0
    return sorted(widths)


def _swiglu_block(
    tc,
    pools,
    xT_ap,
    n_rows,
    w1_ap,
    w3_ap,
    w2_ap,
    out_ap,
    ge_ap,
    pending,
    use_silu=True,
):
    """Emit one SwiGLU y = (silu(x@w1) * (x@w3)) @ w2 over n_rows tokens.

    xT_ap: [D, n_rows] (transposed activations), w1/w3: [D, H], w2: [H, D],
    out_ap: [n_rows, D]. If ge_ap ([n_rows, 1]) is given, output rows are
    scaled by it. `pending` is the cross-chunk/cross-block queue of deferred
    phase-2 emitters; this block's own phase-2 units are appended to it and
    the caller must flush whatever remains at the end.
    """
    nc = tc.nc
    D = w1_ap.shape[0]
    H = w1_ap.shape[1]
    KD = D // P
    KH = H // P
    ND = D // 512  # output free-dim tiles

    xpool, w2pool, wpool, gpool, spool, opool, gepool, pp1, pp3, ppo = pools

    if ge_ap is not None:
        get_ = gepool.tile([P, n_rows // P], F32, tag="ge", name="get_")
        nc.sync.dma_start(get_[:], ge_ap.rearrange("(c p) one -> p (c one)", p=P))

    xr = xT_ap.rearrange("(k p) n -> p k n", p=P)
    w1r = w1_ap.rearrange("(k p) h -> p k h", p=P)
    w3r = w3_ap.rearrange("(k p) h -> p k h", p=P)
    w2r = w2_ap.rearrange("(k p) d -> p k d", p=P)
    w2cell = [None]  # loaded lazily so startup DMAs prioritize phase-1 operands

    def make_unit(gt, c0, ct, dn, ot_cell):
        """Deferred phase-2 emitter: out[c0+ct*P : +P, dn*512 : +512]."""

        def emit():
            if dn == 0:
                ot_cell[0] = opool.tile([P, D], F32, tag="ot", name="ot")
            ot = ot_cell[0]
            po = ppo.tile([P, 512], F32, tag="po", name="po")
            for kh in range(KH):
                nc.tensor.matmul(
                    po,
                    gt[:, kh, ct * P : (ct + 1) * P],
                    w2cell[0][:, kh, dn * 512 : (dn + 1) * 512],
                    start=(kh == 0),
                    stop=(kh == KH - 1),
                )
            if ge_ap is not None:
                nc.vector.tensor_scalar_mul(
                    ot[:, dn * 512 : (dn + 1) * 512],
                    po,
                    get_[:, c0 // P + ct : c0 // P + ct + 1],
                )
            else:
                nc.vector.tensor_copy(ot[:, dn * 512 : (dn + 1) * 512], po)
            if dn == ND - 1:
                nc.sync.dma_start(out_ap[c0 + ct * P : c0 + (ct + 1) * P, :], ot[:])

        return emit

    c0 = 0
    for cw in _chunk_widths(n_rows):
        # per-chunk activation slice (double-buffered: next chunk prefetches)
        xt = xpool.tile([P, KD, 512], F32R, tag="xT", name="xt")[:, :, :cw]
        nc.sync.dma_start(xt[:], xr[:, :, c0 : c0 + cw])

        # ---- phase 1: gT[h, c] = silu(h1T) * h3T for this chunk ----
        gt = gpool.tile([P, KH, 512], F32R, tag="gT", name="gt")
        for hb in range(KH // H_BLOCK):
            w1t = wpool.tile([P, KD, H_BLOCK * P], F32R, tag="w1t", name="w1t")
            nc.sync.dma_start(
                w1t[:], w1r[:, :, hb * H_BLOCK * P : (hb + 1) * H_BLOCK * P]
            )
            w3t = wpool.tile([P, KD, H_BLOCK * P], F32R, tag="w3t", name="w3t")
            nc.sync.dma_start(
                w3t[:], w3r[:, :, hb * H_BLOCK * P : (hb + 1) * H_BLOCK * P]
            )
            for hi in range(H_BLOCK):
                h = hb * H_BLOCK + hi
                p1 = pp1.tile([P, 512], F32, tag="p1", name="p1")[:, :cw]
                p3 = pp3.tile([P, 512], F32, tag="p3", name="p3")[:, :cw]
                for k in range(KD):
                    nc.tensor.matmul(
                        p1,
                        w1t[:, k, hi * P : (hi + 1) * P],
                        xt[:, k, :],
                        start=(k == 0),
                        stop=(k == KD - 1),
                    )
                for k in range(KD):
                    nc.tensor.matmul(
                        p3,
                        w3t[:, k, hi * P : (hi + 1) * P],
                        xt[:, k, :],
                        start=(k == 0),
                        stop=(k == KD - 1),
                    )
                s1 = spool.tile([P, 512], F32, tag="s1", name="s1")[:, :cw]
                if use_silu:
                    nc.scalar.activation(s1, p1, AF.Silu)
                    nc.vector.tensor_mul(gt[:, h, :cw], s1, p3)
                else:  # silu(a) = a * sigmoid(a); CoreSim has no Silu table
                    nc.scalar.activation(s1, p1, AF.Sigmoid)
                    nc.vector.tensor_mul(gt[:, h, :cw], p1, p3)
                    nc.vector.tensor_mul(gt[:, h, :cw], gt[:, h, :cw], s1)

            # interleave one deferred phase-2 unit (previous chunk / block)
            if pending:
                pending.pop(0)()

        if w2cell[0] is None:
            # emitted after the first phase-1 so startup DMAs aren't stuck
            # behind this 8 MB transfer; needed only once phase 2 begins
            w2cell[0] = w2pool.tile([P, KH, D], F32R, tag="w2res", name="w2t")
            half = KH // 2
            nc.sync.dma_start(w2cell[0][:, :half, :], w2r[:, :half, :])
            nc.sync.dma_start(w2cell[0][:, half:, :], w2r[:, half:, :])

        # queue this chunk's phase-2 work
        for ct in range(cw // P):
            ot_cell = [None]
            for dn in range(ND):
                pending.append(make_unit(gt, c0, ct, dn, ot_cell))
        c0 += cw


def build_moe_program(D, H, C, S, use_silu=True):
    """SPMD program: routed expert over C capacity rows + shared expert over
    S token-slice rows. Same NEFF on all 8 cores, per-core input data."""
    nc = bacc.Bacc(
        "TRN2", target_bir_lowering=False, debug=False, num_devices=N_CORES
    )

    def din(name, shape, dt=F32):
        return nc.dram_tensor(name, shape, dt, kind="ExternalInput").ap()

    def dout(name, shape):
        return nc.dram_tensor(name, shape, F32, kind="ExternalOutput").ap()

    xeT = din("xeT", [D, C], F32R)
    ge = din("ge", [C, 1])
    xsT = din("xsT", [D, S], F32R)
    w1 = din("w1", [D, H], F32R)
    w3 = din("w3", [D, H], F32R)
    w2 = din("w2", [H, D], F32R)
    sw1 = din("sw1", [D, H], F32R)
    sw3 = din("sw3", [D, H], F32R)
    sw2 = din("sw2", [H, D], F32R)
    ye = dout("ye", [C, D])
    se = dout("se", [S, D])

    with tile.TileContext(nc) as tc:
        from contextlib import ExitStack

        with ExitStack() as ctx:
            pools = (
                ctx.enter_context(tc.tile_pool(name="xT", bufs=2)),
                ctx.enter_context(tc.tile_pool(name="w2res", bufs=1)),
                ctx.enter_context(tc.tile_pool(name="wstream", bufs=2)),
                ctx.enter_context(tc.tile_pool(name="gT", bufs=2)),
                ctx.enter_context(tc.tile_pool(name="stemp", bufs=2)),
                ctx.enter_context(tc.tile_pool(name="otile", bufs=2)),
                ctx.enter_context(tc.tile_pool(name="gate", bufs=1)),
                ctx.enter_context(tc.tile_pool(name="ps1", bufs=2, space="PSUM")),
                ctx.enter_context(tc.tile_pool(name="ps3", bufs=2, space="PSUM")),
                ctx.enter_context(tc.tile_pool(name="pso", bufs=2, space="PSUM")),
            )
            pending = []
            _swiglu_block(
                tc, pools, xeT, C, w1, w3, w2, ye, ge, pending, use_silu
            )
            _swiglu_block(
                tc, pools, xsT, S, sw1, sw3, sw2, se, None, pending, use_silu
            )
            for unit in pending:
                unit()

    nc.compile()
    return nc


_PROGRAM_CACHE = {}
LAST_RESULTS = None  # BassKernelResults of the most recent device run (for test.py)


def _get_program(D, H, C, S):
    key = (D, H, C, S)
    if key not in _PROGRAM_CACHE:
        _PROGRAM_CACHE[key] = build_moe_program(D, H, C, S)
    return _PROGRAM_CACHE[key]


def _route(xf, w_router):
    """Top-2 routing identical (up to fp rounding) to the jax reference."""
    logits = xf @ w_router.astype(np.float32)  # [T, E]
    # softmax is monotone: top-2 of probs == top-2 of logits, stable ties
    top2 = np.argsort(-logits, axis=1, kind="stable")[:, :2]  # [T, 2]
    lv = np.take_along_axis(logits, top2, axis=1)
    ev = np.exp(lv - lv[:, 0:1])
    gates = ev / ev.sum(axis=1, keepdims=True)  # [T, 2] renormalized
    return top2, gates


def kernel(x, w_router, w1, w3, w2, sw1, sw3, sw2):
    B, SEQ, D = x.shape
    T = B * SEQ
    E, _, H = w1.shape
    assert E == N_CORES
    S = T // N_CORES

    x = np.asarray(x, dtype=np.float32)
    xf = np.ascontiguousarray(x.reshape(T, D))
    top2, gates = _route(xf, np.asarray(w_router, np.float32))

    # per-expert token lists + gate values
    flat_e = top2.ravel()  # slot 2t, 2t+1 -> token t
    flat_g = gates.ravel().astype(np.float32)
    order = np.argsort(flat_e, kind="stable")
    sorted_e = flat_e[order]
    starts = np.searchsorted(sorted_e, np.arange(E + 1))
    tok_by_e = [order[starts[e] : starts[e + 1]] >> 1 for e in range(E)]
    gate_by_e = [flat_g[order[starts[e] : starts[e + 1]]] for e in range(E)]
    counts = np.diff(starts)

    # capacity: fixed floor so the compiled program is reused across calls
    C = max(1152, (int(counts.max()) + 127) // 128 * 128)

    nc = _get_program(D, H, C, S)

    w1 = np.asarray(w1, np.float32)
    w3 = np.asarray(w3, np.float32)
    w2 = np.asarray(w2, np.float32)
    sw1 = np.ascontiguousarray(np.asarray(sw1, np.float32))
    sw3 = np.ascontiguousarray(np.asarray(sw3, np.float32))
    sw2 = np.ascontiguousarray(np.asarray(sw2, np.float32))

    in_maps = []
    for e in range(E):
        n_e = int(counts[e])
        xeT = np.zeros((D, C), np.float32)
        xeT[:, :n_e] = xf[tok_by_e[e]].T
        ge = np.zeros((C, 1), np.float32)
        ge[:n_e, 0] = gate_by_e[e]
        xsT = np.ascontiguousarray(xf[e * S : (e + 1) * S].T)
        in_maps.append(
            {
                "xeT": xeT,
                "ge": ge,
                "xsT": xsT,
                "w1": np.ascontiguousarray(w1[e]),
                "w3": np.ascontiguousarray(w3[e]),
                "w2": np.ascontiguousarray(w2[e]),
                "sw1": sw1,
                "sw3": sw3,
                "sw2": sw2,
            }
        )

    global LAST_RESULTS
    LAST_RESULTS = run_bass_kernel_spmd(nc, in_maps, core_ids=list(range(N_CORES)))
    res = LAST_RESULTS.results

    out = np.empty((T, D), np.float32)
    for c in range(N_CORES):
        out[c * S : (c + 1) * S] = res[c]["se"]
    for e in range(E):
        n_e = int(counts[e])
        if n_e:
            out[tok_by_e[e]] += res[e]["ye"][:n_e]
    return out.reshape(B, SEQ, D)


# revision 16
# speedup vs baseline: 1.0686x; 1.0686x over previous
"""MoE feed-forward (top-2 routing + shared expert) on 8 Trainium2 cores.

Strategy (expert parallel):
  - Host computes the router (tiny [T,D]@[D,E] matmul), top-2 expert ids and
    renormalized gates, then dispatches each expert's tokens (transposed,
    capacity-padded) to the core that owns that expert's weights.
  - Core e computes  ye = (silu(xe@w1_e) * (xe@w3_e)) @ w2_e, row-scaled by the
    gate, plus a 1/8 token-slice of the always-active shared expert.
  - Host scatter-adds routed outputs into the shared-expert output.

On-device matmuls use float32r (fp32 data with fp22 multiplies, fp32
accumulation) which runs the PE at full rate for free dims >= 256.

Pipeline shape: tokens are processed in chunks of <=512 columns. Phase 1
(h1/h3/swiglu-gate, weight-streaming, DMA-heavy) of chunk i+1 is interleaved
with phase 2 (down-projection, DMA-light) of chunk i via a pending-unit queue,
smoothing DMA demand and keeping the PE stream dense.
"""

import numpy as np

import concourse.bass as bass
import concourse.mybir as mybir
import concourse.tile as tile
from concourse import bacc
from concourse.bass_utils import run_bass_kernel_spmd

P = 128
N_CORES = 8
F32 = mybir.dt.float32
F32R = mybir.dt.float32r
AF = mybir.ActivationFunctionType

# h-tiles of w1/w3 fetched per DMA (bigger transfers, fewer descriptors)
H_BLOCK = 2


def _chunk_widths(n):
    """Split n (multiple of 128) into widths of 256..512 (multiples of 128):
    float32r matmuls run at full PE rate only for free dim >= 256.
    Widest chunk first: phase-1 weight traffic per chunk is constant, so wide
    chunks (more PE time per chunk) keep the DMA stream ahead of the PE."""
    assert n % P == 0
    if n < 2 * P:
        return [n]
    widths = []
    while n > 0:
        if n >= 640 or n == 512:
            widths.append(512)
            n -= 512
        else:  # 256, 384 (n==640 handled above)
            widths.append(n)
            n = 0
    return widths


def _swiglu_block(
    tc,
    pools,
    xT_ap,
    n_rows,
    w1_ap,
    w3_ap,
    w2_ap,
    out_ap,
    ge_ap,
    pending,
    use_silu=True,
):
    """Emit one SwiGLU y = (silu(x@w1) * (x@w3)) @ w2 over n_rows tokens.

    xT_ap: [D, n_rows] (transposed activations), w1/w3: [D, H], w2: [H, D],
    out_ap: [n_rows, D]. If ge_ap ([n_rows, 1]) is given, output rows are
    scaled by it. `pending` is the cross-chunk/cross-block queue of deferred
    phase-2 emitters; this block's own phase-2 units are appended to it and
    the caller must flush whatever remains at the end.
    """
    nc = tc.nc
    D = w1_ap.shape[0]
    H = w1_ap.shape[1]
    KD = D // P
    KH = H // P
    ND = D // 512  # output free-dim tiles

    xpool, w2pool, wpool, gpool, spool, opool, gepool, pp1, pp3, ppo = pools

    if ge_ap is not None:
        get_ = gepool.tile([P, n_rows // P], F32, tag="ge", name="get_")
        nc.sync.dma_start(get_[:], ge_ap.rearrange("(c p) one -> p (c one)", p=P))

    xr = xT_ap.rearrange("(k p) n -> p k n", p=P)
    w1r = w1_ap.rearrange("(k p) h -> p k h", p=P)
    w3r = w3_ap.rearrange("(k p) h -> p k h", p=P)
    w2r = w2_ap.rearrange("(k p) d -> p k d", p=P)
    # loaded lazily in halves so startup DMAs prioritize phase-1 operands
    w2cell = [None, False]  # [tile, both_halves_staged]

    def _stage_w2():
        if w2cell[0] is None:
            w2cell[0] = w2pool.tile([P, KH, D], F32R, tag="w2res", name="w2t")
            nc.sync.dma_start(w2cell[0][:, : KH // 2, :], w2r[:, : KH // 2, :])
        elif not w2cell[1]:
            nc.sync.dma_start(w2cell[0][:, KH // 2 :, :], w2r[:, KH // 2 :, :])
            w2cell[1] = True

    def make_unit(gt, c0, ct, dn, ot_cell):
        """Deferred phase-2 emitter: out[c0+ct*P : +P, dn*512 : +512]."""

        def emit():
            if dn == 0:
                ot_cell[0] = opool.tile([P, D], F32, tag="ot", name="ot")
            ot = ot_cell[0]
            po = ppo.tile([P, 512], F32, tag="po", name="po")
            for kh in range(KH):
                nc.tensor.matmul(
                    po,
                    gt[:, kh, ct * P : (ct + 1) * P],
                    w2cell[0][:, kh, dn * 512 : (dn + 1) * 512],
                    start=(kh == 0),
                    stop=(kh == KH - 1),
                )
            if ge_ap is not None:
                nc.vector.tensor_scalar_mul(
                    ot[:, dn * 512 : (dn + 1) * 512],
                    po,
                    get_[:, c0 // P + ct : c0 // P + ct + 1],
                )
            else:
                nc.vector.tensor_copy(ot[:, dn * 512 : (dn + 1) * 512], po)
            if dn == ND - 1:
                nc.sync.dma_start(out_ap[c0 + ct * P : c0 + (ct + 1) * P, :], ot[:])

        return emit

    c0 = 0
    for cw in _chunk_widths(n_rows):
        # per-chunk activation slice (double-buffered: next chunk prefetches)
        xt = xpool.tile([P, KD, 512], F32R, tag="xT", name="xt")[:, :, :cw]
        nc.sync.dma_start(xt[:], xr[:, :, c0 : c0 + cw])

        # ---- phase 1: gT[h, c] = silu(h1T) * h3T for this chunk ----
        gt = gpool.tile([P, KH, 512], F32R, tag="gT", name="gt")
        for hb in range(KH // H_BLOCK):
            w1t = wpool.tile([P, KD, H_BLOCK * P], F32R, tag="w1t", name="w1t")
            nc.sync.dma_start(
                w1t[:], w1r[:, :, hb * H_BLOCK * P : (hb + 1) * H_BLOCK * P]
            )
            w3t = wpool.tile([P, KD, H_BLOCK * P], F32R, tag="w3t", name="w3t")
            nc.sync.dma_start(
                w3t[:], w3r[:, :, hb * H_BLOCK * P : (hb + 1) * H_BLOCK * P]
            )
            for hi in range(H_BLOCK):
                h = hb * H_BLOCK + hi
                p1 = pp1.tile([P, 512], F32, tag="p1", name="p1")[:, :cw]
                p3 = pp3.tile([P, 512], F32, tag="p3", name="p3")[:, :cw]
                for k in range(KD):
                    nc.tensor.matmul(
                        p1,
                        w1t[:, k, hi * P : (hi + 1) * P],
                        xt[:, k, :],
                        start=(k == 0),
                        stop=(k == KD - 1),
                    )
                for k in range(KD):
                    nc.tensor.matmul(
                        p3,
                        w3t[:, k, hi * P : (hi + 1) * P],
                        xt[:, k, :],
                        start=(k == 0),
                        stop=(k == KD - 1),
                    )
                s1 = spool.tile([P, 512], F32, tag="s1", name="s1")[:, :cw]
                if use_silu:
                    nc.scalar.activation(s1, p1, AF.Silu)
                    nc.vector.tensor_mul(gt[:, h, :cw], s1, p3)
                else:  # silu(a) = a * sigmoid(a); CoreSim has no Silu table
                    nc.scalar.activation(s1, p1, AF.Sigmoid)
                    nc.vector.tensor_mul(gt[:, h, :cw], p1, p3)
                    nc.vector.tensor_mul(gt[:, h, :cw], gt[:, h, :cw], s1)

            # stage the down-projection weights behind the first few phase-1
            # weight tiles; needed only once this block's phase 2 begins
            if hb in (1, 3):
                _stage_w2()

            # interleave one deferred phase-2 unit (previous chunk / block)
            if pending:
                pending.pop(0)()

        while not w2cell[1]:  # small configs: ensure both halves staged
            _stage_w2()

        # queue this chunk's phase-2 work
        for ct in range(cw // P):
            ot_cell = [None]
            for dn in range(ND):
                pending.append(make_unit(gt, c0, ct, dn, ot_cell))
        c0 += cw


def build_moe_program(D, H, C, S, use_silu=True):
    """SPMD program: routed expert over C capacity rows + shared expert over
    S token-slice rows. Same NEFF on all 8 cores, per-core input data."""
    nc = bacc.Bacc(
        "TRN2", target_bir_lowering=False, debug=False, num_devices=N_CORES
    )

    def din(name, shape, dt=F32):
        return nc.dram_tensor(name, shape, dt, kind="ExternalInput").ap()

    def dout(name, shape):
        return nc.dram_tensor(name, shape, F32, kind="ExternalOutput").ap()

    xeT = din("xeT", [D, C], F32R)
    ge = din("ge", [C, 1])
    xsT = din("xsT", [D, S], F32R)
    w1 = din("w1", [D, H], F32R)
    w3 = din("w3", [D, H], F32R)
    w2 = din("w2", [H, D], F32R)
    sw1 = din("sw1", [D, H], F32R)
    sw3 = din("sw3", [D, H], F32R)
    sw2 = din("sw2", [H, D], F32R)
    ye = dout("ye", [C, D])
    se = dout("se", [S, D])

    with tile.TileContext(nc) as tc:
        from contextlib import ExitStack

        with ExitStack() as ctx:
            pools = (
                ctx.enter_context(tc.tile_pool(name="xT", bufs=2)),
                ctx.enter_context(tc.tile_pool(name="w2res", bufs=1)),
                ctx.enter_context(tc.tile_pool(name="wstream", bufs=2)),
                ctx.enter_context(tc.tile_pool(name="gT", bufs=2)),
                ctx.enter_context(tc.tile_pool(name="stemp", bufs=2)),
                ctx.enter_context(tc.tile_pool(name="otile", bufs=2)),
                ctx.enter_context(tc.tile_pool(name="gate", bufs=1)),
                ctx.enter_context(tc.tile_pool(name="ps1", bufs=2, space="PSUM")),
                ctx.enter_context(tc.tile_pool(name="ps3", bufs=2, space="PSUM")),
                ctx.enter_context(tc.tile_pool(name="pso", bufs=2, space="PSUM")),
            )
            pending = []
            _swiglu_block(
                tc, pools, xeT, C, w1, w3, w2, ye, ge, pending, use_silu
            )
            _swiglu_block(
                tc, pools, xsT, S, sw1, sw3, sw2, se, None, pending, use_silu
            )
            for unit in pending:
                unit()

    nc.compile()
    return nc


_PROGRAM_CACHE = {}
LAST_RESULTS = None  # BassKernelResults of the most recent device run (for test.py)


def _get_program(D, H, C, S):
    key = (D, H, C, S)
    if key not in _PROGRAM_CACHE:
        _PROGRAM_CACHE[key] = build_moe_program(D, H, C, S)
    return _PROGRAM_CACHE[key]


def _route(xf, w_router):
    """Top-2 routing identical (up to fp rounding) to the jax reference."""
    logits = xf @ w_router.astype(np.float32)  # [T, E]
    # softmax is monotone: top-2 of probs == top-2 of logits, stable ties
    top2 = np.argsort(-logits, axis=1, kind="stable")[:, :2]  # [T, 2]
    lv = np.take_along_axis(logits, top2, axis=1)
    ev = np.exp(lv - lv[:, 0:1])
    gates = ev / ev.sum(axis=1, keepdims=True)  # [T, 2] renormalized
    return top2, gates


def kernel(x, w_router, w1, w3, w2, sw1, sw3, sw2):
    B, SEQ, D = x.shape
    T = B * SEQ
    E, _, H = w1.shape
    assert E == N_CORES
    S = T // N_CORES

    x = np.asarray(x, dtype=np.float32)
    xf = np.ascontiguousarray(x.reshape(T, D))
    top2, gates = _route(xf, np.asarray(w_router, np.float32))

    # per-expert token lists + gate values
    flat_e = top2.ravel()  # slot 2t, 2t+1 -> token t
    flat_g = gates.ravel().astype(np.float32)
    order = np.argsort(flat_e, kind="stable")
    sorted_e = flat_e[order]
    starts = np.searchsorted(sorted_e, np.arange(E + 1))
    tok_by_e = [order[starts[e] : starts[e + 1]] >> 1 for e in range(E)]
    gate_by_e = [flat_g[order[starts[e] : starts[e + 1]]] for e in range(E)]
    counts = np.diff(starts)

    # capacity: fixed floor so the compiled program is reused across calls
    C = max(1152, (int(counts.max()) + 127) // 128 * 128)

    nc = _get_program(D, H, C, S)

    w1 = np.asarray(w1, np.float32)
    w3 = np.asarray(w3, np.float32)
    w2 = np.asarray(w2, np.float32)
    sw1 = np.ascontiguousarray(np.asarray(sw1, np.float32))
    sw3 = np.ascontiguousarray(np.asarray(sw3, np.float32))
    sw2 = np.ascontiguousarray(np.asarray(sw2, np.float32))

    in_maps = []
    for e in range(E):
        n_e = int(counts[e])
        xeT = np.zeros((D, C), np.float32)
        xeT[:, :n_e] = xf[tok_by_e[e]].T
        ge = np.zeros((C, 1), np.float32)
        ge[:n_e, 0] = gate_by_e[e]
        xsT = np.ascontiguousarray(xf[e * S : (e + 1) * S].T)
        in_maps.append(
            {
                "xeT": xeT,
                "ge": ge,
                "xsT": xsT,
                "w1": np.ascontiguousarray(w1[e]),
                "w3": np.ascontiguousarray(w3[e]),
                "w2": np.ascontiguousarray(w2[e]),
                "sw1": sw1,
                "sw3": sw3,
                "sw2": sw2,
            }
        )

    global LAST_RESULTS
    LAST_RESULTS = run_bass_kernel_spmd(nc, in_maps, core_ids=list(range(N_CORES)))
    res = LAST_RESULTS.results

    out = np.empty((T, D), np.float32)
    for c in range(N_CORES):
        out[c * S : (c + 1) * S] = res[c]["se"]
    for e in range(E):
        n_e = int(counts[e])
        if n_e:
            out[tok_by_e[e]] += res[e]["ye"][:n_e]
    return out.reshape(B, SEQ, D)


# revision 17
# speedup vs baseline: 1.1167x; 1.0450x over previous
"""MoE feed-forward (top-2 routing + shared expert) on 8 Trainium2 cores.

Strategy (expert parallel):
  - Host computes the router (tiny [T,D]@[D,E] matmul), top-2 expert ids and
    renormalized gates, then dispatches each expert's tokens (transposed,
    capacity-padded) to the core that owns that expert's weights.
  - Core e computes  ye = (silu(xe@w1_e) * (xe@w3_e)) @ w2_e, row-scaled by the
    gate, plus a 1/8 token-slice of the always-active shared expert.
  - Host scatter-adds routed outputs into the shared-expert output.

On-device matmuls use float32r (fp32 data with fp22 multiplies, fp32
accumulation) which runs the PE at full rate for free dims >= 256.

Pipeline shape: tokens are processed in chunks of <=512 columns. Phase 1
(h1/h3/swiglu-gate, weight-streaming, DMA-heavy) of chunk i+1 is interleaved
with phase 2 (down-projection, DMA-light) of chunk i via a pending-unit queue,
smoothing DMA demand and keeping the PE stream dense.
"""

import numpy as np

import concourse.bass as bass
import concourse.mybir as mybir
import concourse.tile as tile
from concourse import bacc
from concourse.bass_utils import run_bass_kernel_spmd

P = 128
N_CORES = 8
F32 = mybir.dt.float32
F32R = mybir.dt.float32r
AF = mybir.ActivationFunctionType

# h-tiles of w1/w3 fetched per DMA (bigger transfers, fewer descriptors)
H_BLOCK = 2


def _chunk_widths(n):
    """Split n (multiple of 128) into widths of 256..512 (multiples of 128):
    float32r matmuls run at full PE rate only for free dim >= 256.
    Widest chunk first: phase-1 weight traffic per chunk is constant, so wide
    chunks (more PE time per chunk) keep the DMA stream ahead of the PE."""
    assert n % P == 0
    if n < 2 * P:
        return [n]
    widths = []
    while n > 0:
        if n >= 640 or n == 512:
            widths.append(512)
            n -= 512
        else:  # 256, 384 (n==640 handled above)
            widths.append(n)
            n = 0
    return widths


def _swiglu_block(
    tc,
    pools,
    xT_ap,
    n_rows,
    w1_ap,
    w3_ap,
    w2_ap,
    out_ap,
    ge_ap,
    pending,
    use_silu=True,
):
    """Emit one SwiGLU y = (silu(x@w1) * (x@w3)) @ w2 over n_rows tokens.

    xT_ap: [D, n_rows] (transposed activations), w1/w3: [D, H], w2: [H, D],
    out_ap: [n_rows, D]. If ge_ap ([n_rows, 1]) is given, output rows are
    scaled by it. `pending` is the cross-chunk/cross-block queue of deferred
    phase-2 emitters; this block's own phase-2 units are appended to it and
    the caller must flush whatever remains at the end.
    """
    nc = tc.nc
    D = w1_ap.shape[0]
    H = w1_ap.shape[1]
    KD = D // P
    KH = H // P
    ND = D // 512  # output free-dim tiles

    xpool, w2pool, wpool, gpool, spool, opool, gepool, pp1, pp3, ppo = pools

    gecell = [None]

    def _get_gate():
        if gecell[0] is None:
            gecell[0] = gepool.tile([P, n_rows // P], F32, tag="ge", name="get_")
            nc.sync.dma_start(
                gecell[0][:], ge_ap.rearrange("(c p) one -> p (c one)", p=P)
            )
        return gecell[0]

    xr = xT_ap.rearrange("(k p) n -> p k n", p=P)
    w1r = w1_ap.rearrange("(k p) h -> p k h", p=P)
    w3r = w3_ap.rearrange("(k p) h -> p k h", p=P)
    w2r = w2_ap.rearrange("(k p) d -> p k d", p=P)
    w2cell = [None]  # loaded lazily so startup DMAs prioritize phase-1 operands

    def _stage_w2():
        # quartered loads: phase 2's kh-progression unblocks as pieces land
        if w2cell[0] is None:
            w2cell[0] = w2pool.tile([P, KH, D], F32R, tag="w2res", name="w2t")
            n_parts = min(4, KH)
            step = KH // n_parts
            for q in range(n_parts):
                nc.sync.dma_start(
                    w2cell[0][:, q * step : (q + 1) * step, :],
                    w2r[:, q * step : (q + 1) * step, :],
                )

    def make_unit(gt, c0, ct, dn, ot_cell):
        """Deferred phase-2 emitter: out[c0+ct*P : +P, dn*512 : +512]."""

        def emit():
            if dn == 0:
                ot_cell[0] = opool.tile([P, D], F32, tag="ot", name="ot")
            ot = ot_cell[0]
            po = ppo.tile([P, 512], F32, tag="po", name="po")
            for kh in range(KH):
                nc.tensor.matmul(
                    po,
                    gt[:, kh, ct * P : (ct + 1) * P],
                    w2cell[0][:, kh, dn * 512 : (dn + 1) * 512],
                    start=(kh == 0),
                    stop=(kh == KH - 1),
                )
            if ge_ap is not None:
                nc.vector.tensor_scalar_mul(
                    ot[:, dn * 512 : (dn + 1) * 512],
                    po,
                    _get_gate()[:, c0 // P + ct : c0 // P + ct + 1],
                )
            else:
                nc.vector.tensor_copy(ot[:, dn * 512 : (dn + 1) * 512], po)
            if dn == ND - 1:
                nc.sync.dma_start(out_ap[c0 + ct * P : c0 + (ct + 1) * P, :], ot[:])

        return emit

    c0 = 0
    for cw in _chunk_widths(n_rows):
        # per-chunk activation slice (double-buffered: next chunk prefetches)
        xt = xpool.tile([P, KD, 512], F32R, tag="xT", name="xt")[:, :, :cw]
        kh2 = KD // 2
        nc.sync.dma_start(xt[:, :kh2, :], xr[:, :kh2, c0 : c0 + cw])
        nc.sync.dma_start(xt[:, kh2:, :], xr[:, kh2:, c0 : c0 + cw])

        # ---- phase 1: gT[h, c] = silu(h1T) * h3T for this chunk ----
        gt = gpool.tile([P, KH, 512], F32R, tag="gT", name="gt")
        for hb in range(KH // H_BLOCK):
            w1t = wpool.tile([P, KD, H_BLOCK * P], F32R, tag="w1t", name="w1t")
            nc.sync.dma_start(
                w1t[:], w1r[:, :, hb * H_BLOCK * P : (hb + 1) * H_BLOCK * P]
            )
            w3t = wpool.tile([P, KD, H_BLOCK * P], F32R, tag="w3t", name="w3t")
            nc.sync.dma_start(
                w3t[:], w3r[:, :, hb * H_BLOCK * P : (hb + 1) * H_BLOCK * P]
            )
            for hi in range(H_BLOCK):
                h = hb * H_BLOCK + hi
                p1 = pp1.tile([P, 512], F32, tag="p1", name="p1")[:, :cw]
                p3 = pp3.tile([P, 512], F32, tag="p3", name="p3")[:, :cw]
                for k in range(KD):
                    nc.tensor.matmul(
                        p1,
                        w1t[:, k, hi * P : (hi + 1) * P],
                        xt[:, k, :],
                        start=(k == 0),
                        stop=(k == KD - 1),
                    )
                for k in range(KD):
                    nc.tensor.matmul(
                        p3,
                        w3t[:, k, hi * P : (hi + 1) * P],
                        xt[:, k, :],
                        start=(k == 0),
                        stop=(k == KD - 1),
                    )
                s1 = spool.tile([P, 512], F32, tag="s1", name="s1")[:, :cw]
                if use_silu:
                    nc.scalar.activation(s1, p1, AF.Silu)
                    nc.vector.tensor_mul(gt[:, h, :cw], s1, p3)
                else:  # silu(a) = a * sigmoid(a); CoreSim has no Silu table
                    nc.scalar.activation(s1, p1, AF.Sigmoid)
                    nc.vector.tensor_mul(gt[:, h, :cw], p1, p3)
                    nc.vector.tensor_mul(gt[:, h, :cw], gt[:, h, :cw], s1)

        _stage_w2()

        # ---- phase 2 for this chunk ----
        for ct in range(cw // P):
            ot_cell = [None]
            for dn in range(ND):
                make_unit(gt, c0, ct, dn, ot_cell)()
        c0 += cw


def build_moe_program(D, H, C, S, use_silu=True):
    """SPMD program: routed expert over C capacity rows + shared expert over
    S token-slice rows. Same NEFF on all 8 cores, per-core input data."""
    nc = bacc.Bacc(
        "TRN2", target_bir_lowering=False, debug=False, num_devices=N_CORES
    )

    def din(name, shape, dt=F32):
        return nc.dram_tensor(name, shape, dt, kind="ExternalInput").ap()

    def dout(name, shape):
        return nc.dram_tensor(name, shape, F32, kind="ExternalOutput").ap()

    xeT = din("xeT", [D, C], F32R)
    ge = din("ge", [C, 1])
    xsT = din("xsT", [D, S], F32R)
    w1 = din("w1", [D, H], F32R)
    w3 = din("w3", [D, H], F32R)
    w2 = din("w2", [H, D], F32R)
    sw1 = din("sw1", [D, H], F32R)
    sw3 = din("sw3", [D, H], F32R)
    sw2 = din("sw2", [H, D], F32R)
    ye = dout("ye", [C, D])
    se = dout("se", [S, D])

    with tile.TileContext(nc) as tc:
        from contextlib import ExitStack

        with ExitStack() as ctx:
            pools = (
                ctx.enter_context(tc.tile_pool(name="xT", bufs=2)),
                ctx.enter_context(tc.tile_pool(name="w2res", bufs=1)),
                ctx.enter_context(tc.tile_pool(name="wstream", bufs=3)),
                ctx.enter_context(tc.tile_pool(name="gT", bufs=1)),
                ctx.enter_context(tc.tile_pool(name="stemp", bufs=2)),
                ctx.enter_context(tc.tile_pool(name="otile", bufs=2)),
                ctx.enter_context(tc.tile_pool(name="gate", bufs=1)),
                ctx.enter_context(tc.tile_pool(name="ps1", bufs=2, space="PSUM")),
                ctx.enter_context(tc.tile_pool(name="ps3", bufs=2, space="PSUM")),
                ctx.enter_context(tc.tile_pool(name="pso", bufs=2, space="PSUM")),
            )
            pending = []
            _swiglu_block(
                tc, pools, xeT, C, w1, w3, w2, ye, ge, pending, use_silu
            )
            _swiglu_block(
                tc, pools, xsT, S, sw1, sw3, sw2, se, None, pending, use_silu
            )
            for unit in pending:
                unit()

    nc.compile()
    return nc


_PROGRAM_CACHE = {}
LAST_RESULTS = None  # BassKernelResults of the most recent device run (for test.py)


def _get_program(D, H, C, S):
    key = (D, H, C, S)
    if key not in _PROGRAM_CACHE:
        _PROGRAM_CACHE[key] = build_moe_program(D, H, C, S)
    return _PROGRAM_CACHE[key]


def _route(xf, w_router):
    """Top-2 routing identical (up to fp rounding) to the jax reference."""
    logits = xf @ w_router.astype(np.float32)  # [T, E]
    # softmax is monotone: top-2 of probs == top-2 of logits, stable ties
    top2 = np.argsort(-logits, axis=1, kind="stable")[:, :2]  # [T, 2]
    lv = np.take_along_axis(logits, top2, axis=1)
    ev = np.exp(lv - lv[:, 0:1])
    gates = ev / ev.sum(axis=1, keepdims=True)  # [T, 2] renormalized
    return top2, gates


def kernel(x, w_router, w1, w3, w2, sw1, sw3, sw2):
    B, SEQ, D = x.shape
    T = B * SEQ
    E, _, H = w1.shape
    assert E == N_CORES
    S = T // N_CORES

    x = np.asarray(x, dtype=np.float32)
    xf = np.ascontiguousarray(x.reshape(T, D))
    top2, gates = _route(xf, np.asarray(w_router, np.float32))

    # per-expert token lists + gate values
    flat_e = top2.ravel()  # slot 2t, 2t+1 -> token t
    flat_g = gates.ravel().astype(np.float32)
    order = np.argsort(flat_e, kind="stable")
    sorted_e = flat_e[order]
    starts = np.searchsorted(sorted_e, np.arange(E + 1))
    tok_by_e = [order[starts[e] : starts[e + 1]] >> 1 for e in range(E)]
    gate_by_e = [flat_g[order[starts[e] : starts[e + 1]]] for e in range(E)]
    counts = np.diff(starts)

    # capacity: fixed floor so the compiled program is reused across calls
    C = max(1152, (int(counts.max()) + 127) // 128 * 128)

    nc = _get_program(D, H, C, S)

    w1 = np.asarray(w1, np.float32)
    w3 = np.asarray(w3, np.float32)
    w2 = np.asarray(w2, np.float32)
    sw1 = np.ascontiguousarray(np.asarray(sw1, np.float32))
    sw3 = np.ascontiguousarray(np.asarray(sw3, np.float32))
    sw2 = np.ascontiguousarray(np.asarray(sw2, np.float32))

    in_maps = []
    for e in range(E):
        n_e = int(counts[e])
        xeT = np.zeros((D, C), np.float32)
        xeT[:, :n_e] = xf[tok_by_e[e]].T
        ge = np.zeros((C, 1), np.float32)
        ge[:n_e, 0] = gate_by_e[e]
        xsT = np.ascontiguousarray(xf[e * S : (e + 1) * S].T)
        in_maps.append(
            {
                "xeT": xeT,
                "ge": ge,
                "xsT": xsT,
                "w1": np.ascontiguousarray(w1[e]),
                "w3": np.ascontiguousarray(w3[e]),
                "w2": np.ascontiguousarray(w2[e]),
                "sw1": sw1,
                "sw3": sw3,
                "sw2": sw2,
            }
        )

    global LAST_RESULTS
    LAST_RESULTS = run_bass_kernel_spmd(nc, in_maps, core_ids=list(range(N_CORES)))
    res = LAST_RESULTS.results

    out = np.empty((T, D), np.float32)
    for c in range(N_CORES):
        out[c * S : (c + 1) * S] = res[c]["se"]
    for e in range(E):
        n_e = int(counts[e])
        if n_e:
            out[tok_by_e[e]] += res[e]["ye"][:n_e]
    return out.reshape(B, SEQ, D)


# revision 18
# speedup vs baseline: 1.1558x; 1.0350x over previous
"""MoE feed-forward (top-2 routing + shared expert) on 8 Trainium2 cores.

Strategy (expert parallel):
  - Host computes the router (tiny [T,D]@[D,E] matmul), top-2 expert ids and
    renormalized gates, then dispatches each expert's tokens (transposed,
    capacity-padded) to the core that owns that expert's weights.
  - Core e computes  ye = (silu(xe@w1_e) * (xe@w3_e)) @ w2_e, row-scaled by the
    gate, plus a 1/8 token-slice of the always-active shared expert.
  - Host scatter-adds routed outputs into the shared-expert output.

On-device matmuls use float32r (fp32 data with fp22 multiplies, fp32
accumulation) which runs the PE at full rate for free dims >= 256.

Pipeline shape: tokens are processed in chunks of <=512 columns. Phase 1
(h1/h3/swiglu-gate, weight-streaming, DMA-heavy) of chunk i+1 is interleaved
with phase 2 (down-projection, DMA-light) of chunk i via a pending-unit queue,
smoothing DMA demand and keeping the PE stream dense.
"""

import numpy as np

import concourse.bass as bass
import concourse.mybir as mybir
import concourse.tile as tile
from concourse import bacc
from concourse.bass_utils import run_bass_kernel_spmd

P = 128
N_CORES = 8
F32 = mybir.dt.float32
F32R = mybir.dt.float32r
AF = mybir.ActivationFunctionType

# h-tiles of w1/w3 fetched per DMA (bigger transfers, fewer descriptors)
H_BLOCK = 2


def _chunk_widths(n):
    """Split n (multiple of 128) into widths of 256..512 (multiples of 128):
    float32r matmuls run at full PE rate only for free dim >= 256.
    Widest chunk first: phase-1 weight traffic per chunk is constant, so wide
    chunks (more PE time per chunk) keep the DMA stream ahead of the PE."""
    assert n % P == 0
    if n < 2 * P:
        return [n]
    widths = []
    while n > 0:
        if n >= 640 or n == 512:
            widths.append(512)
            n -= 512
        else:  # 256, 384 (n==640 handled above)
            widths.append(n)
            n = 0
    return widths


def _swiglu_block(
    tc,
    pools,
    xT_ap,
    n_rows,
    w1_ap,
    w3_ap,
    w2_ap,
    out_ap,
    ge_ap,
    pending,
    use_silu=True,
):
    """Emit one SwiGLU y = (silu(x@w1) * (x@w3)) @ w2 over n_rows tokens.

    xT_ap: [P, (D//P)*n_rows] pre-packed activations (see _pack_xT),
    out_ap: [n_rows, D]. If ge_ap ([n_rows, 1]) is given, output rows are
    scaled by it. `pending` is the cross-chunk/cross-block queue of deferred
    phase-2 emitters; this block's own phase-2 units are appended to it and
    the caller must flush whatever remains at the end.
    """
    nc = tc.nc
    D = w1_ap.shape[0]
    H = w1_ap.shape[1]
    KD = D // P
    KH = H // P
    ND = D // 512  # output free-dim tiles

    xpool, w2pool, wpool, gpool, spool, opool, gepool, pp1, pp3, ppo = pools

    gecell = [None]

    def _get_gate():
        if gecell[0] is None:
            gecell[0] = gepool.tile([P, n_rows // P], F32, tag="ge", name="get_")
            nc.sync.dma_start(
                gecell[0][:], ge_ap.rearrange("(c p) one -> p (c one)", p=P)
            )
        return gecell[0]

    w1r = w1_ap.rearrange("(k p) h -> p k h", p=P)
    w3r = w3_ap.rearrange("(k p) h -> p k h", p=P)
    w2r = w2_ap.rearrange("(k p) d -> p k d", p=P)
    w2cell = [None]  # loaded lazily so startup DMAs prioritize phase-1 operands

    def _stage_w2():
        # quartered loads: phase 2's kh-progression unblocks as pieces land
        if w2cell[0] is None:
            w2cell[0] = w2pool.tile([P, KH, D], F32R, tag="w2res", name="w2t")
            n_parts = min(4, KH)
            step = KH // n_parts
            for q in range(n_parts):
                nc.sync.dma_start(
                    w2cell[0][:, q * step : (q + 1) * step, :],
                    w2r[:, q * step : (q + 1) * step, :],
                )

    def make_unit(gt, c0, ct, dn, ot_cell):
        """Deferred phase-2 emitter: out[c0+ct*P : +P, dn*512 : +512]."""

        def emit():
            if dn == 0:
                ot_cell[0] = opool.tile([P, D], F32, tag="ot", name="ot")
            ot = ot_cell[0]
            po = ppo.tile([P, 512], F32, tag="po", name="po")
            for kh in range(KH):
                nc.tensor.matmul(
                    po,
                    gt[:, kh, ct * P : (ct + 1) * P],
                    w2cell[0][:, kh, dn * 512 : (dn + 1) * 512],
                    start=(kh == 0),
                    stop=(kh == KH - 1),
                )
            if ge_ap is not None:
                nc.vector.tensor_scalar_mul(
                    ot[:, dn * 512 : (dn + 1) * 512],
                    po,
                    _get_gate()[:, c0 // P + ct : c0 // P + ct + 1],
                )
            else:
                nc.vector.tensor_copy(ot[:, dn * 512 : (dn + 1) * 512], po)
            if dn == ND - 1:
                nc.sync.dma_start(out_ap[c0 + ct * P : c0 + (ct + 1) * P, :], ot[:])

        return emit

    c0 = 0
    off = 0
    for cw in _chunk_widths(n_rows):
        # per-chunk activation slice (double-buffered: next chunk prefetches);
        # host packs xT chunk-major so each load is contiguous per partition
        xt = xpool.tile([P, KD, 512], F32R, tag="xT", name="xt")[:, :, :cw]
        xsrc = xT_ap[:, off : off + KD * cw].rearrange("p (k c) -> p k c", k=KD)
        kh2 = KD // 2
        nc.sync.dma_start(xt[:, :kh2, :], xsrc[:, :kh2, :])
        nc.sync.dma_start(xt[:, kh2:, :], xsrc[:, kh2:, :])
        off += KD * cw

        # ---- phase 1: gT[h, c] = silu(h1T) * h3T for this chunk ----
        gt = gpool.tile([P, KH, 512], F32R, tag="gT", name="gt")
        for hb in range(KH // H_BLOCK):
            w1t = wpool.tile([P, KD, H_BLOCK * P], F32R, tag="w1t", name="w1t")
            nc.sync.dma_start(
                w1t[:], w1r[:, :, hb * H_BLOCK * P : (hb + 1) * H_BLOCK * P]
            )
            w3t = wpool.tile([P, KD, H_BLOCK * P], F32R, tag="w3t", name="w3t")
            nc.sync.dma_start(
                w3t[:], w3r[:, :, hb * H_BLOCK * P : (hb + 1) * H_BLOCK * P]
            )
            for hi in range(H_BLOCK):
                h = hb * H_BLOCK + hi
                p1 = pp1.tile([P, 512], F32, tag="p1", name="p1")[:, :cw]
                p3 = pp3.tile([P, 512], F32, tag="p3", name="p3")[:, :cw]
                for k in range(KD):
                    nc.tensor.matmul(
                        p1,
                        w1t[:, k, hi * P : (hi + 1) * P],
                        xt[:, k, :],
                        start=(k == 0),
                        stop=(k == KD - 1),
                    )
                for k in range(KD):
                    nc.tensor.matmul(
                        p3,
                        w3t[:, k, hi * P : (hi + 1) * P],
                        xt[:, k, :],
                        start=(k == 0),
                        stop=(k == KD - 1),
                    )
                s1 = spool.tile([P, 512], F32, tag="s1", name="s1")[:, :cw]
                if use_silu:
                    nc.scalar.activation(s1, p1, AF.Silu)
                    nc.vector.tensor_mul(gt[:, h, :cw], s1, p3)
                else:  # silu(a) = a * sigmoid(a); CoreSim has no Silu table
                    nc.scalar.activation(s1, p1, AF.Sigmoid)
                    nc.vector.tensor_mul(gt[:, h, :cw], p1, p3)
                    nc.vector.tensor_mul(gt[:, h, :cw], gt[:, h, :cw], s1)

        _stage_w2()

        # ---- phase 2 for this chunk ----
        for ct in range(cw // P):
            ot_cell = [None]
            for dn in range(ND):
                make_unit(gt, c0, ct, dn, ot_cell)()
        c0 += cw


def build_moe_program(D, H, C, S, use_silu=True):
    """SPMD program: routed expert over C capacity rows + shared expert over
    S token-slice rows. Same NEFF on all 8 cores, per-core input data."""
    nc = bacc.Bacc(
        "TRN2", target_bir_lowering=False, debug=False, num_devices=N_CORES
    )

    def din(name, shape, dt=F32):
        return nc.dram_tensor(name, shape, dt, kind="ExternalInput").ap()

    def dout(name, shape):
        return nc.dram_tensor(name, shape, F32, kind="ExternalOutput").ap()

    xeT = din("xeT", [P, (D // P) * C], F32R)
    ge = din("ge", [C, 1])
    xsT = din("xsT", [P, (D // P) * S], F32R)
    w1 = din("w1", [D, H], F32R)
    w3 = din("w3", [D, H], F32R)
    w2 = din("w2", [H, D], F32R)
    sw1 = din("sw1", [D, H], F32R)
    sw3 = din("sw3", [D, H], F32R)
    sw2 = din("sw2", [H, D], F32R)
    ye = dout("ye", [C, D])
    se = dout("se", [S, D])

    with tile.TileContext(nc) as tc:
        from contextlib import ExitStack

        with ExitStack() as ctx:
            pools = (
                ctx.enter_context(tc.tile_pool(name="xT", bufs=2)),
                ctx.enter_context(tc.tile_pool(name="w2res", bufs=1)),
                ctx.enter_context(tc.tile_pool(name="wstream", bufs=3)),
                ctx.enter_context(tc.tile_pool(name="gT", bufs=1)),
                ctx.enter_context(tc.tile_pool(name="stemp", bufs=2)),
                ctx.enter_context(tc.tile_pool(name="otile", bufs=2)),
                ctx.enter_context(tc.tile_pool(name="gate", bufs=1)),
                ctx.enter_context(tc.tile_pool(name="ps1", bufs=2, space="PSUM")),
                ctx.enter_context(tc.tile_pool(name="ps3", bufs=2, space="PSUM")),
                ctx.enter_context(tc.tile_pool(name="pso", bufs=2, space="PSUM")),
            )
            pending = []
            _swiglu_block(
                tc, pools, xeT, C, w1, w3, w2, ye, ge, pending, use_silu
            )
            _swiglu_block(
                tc, pools, xsT, S, sw1, sw3, sw2, se, None, pending, use_silu
            )
            for unit in pending:
                unit()

    nc.compile()
    return nc


_PROGRAM_CACHE = {}
LAST_RESULTS = None  # BassKernelResults of the most recent device run (for test.py)


def _get_program(D, H, C, S):
    key = (D, H, C, S)
    if key not in _PROGRAM_CACHE:
        _PROGRAM_CACHE[key] = build_moe_program(D, H, C, S)
    return _PROGRAM_CACHE[key]


def _pack_xT(xmat):
    """[n, D] row-major tokens -> [P, KD*n] partition-major, chunk-contiguous
    layout matching _swiglu_block's per-chunk loads."""
    n, D = xmat.shape
    KD = D // P
    xr = xmat.reshape(n, KD, P).transpose(2, 1, 0)  # [P, KD, n]
    out = np.empty((P, KD * n), np.float32)
    off = 0
    c0 = 0
    for cw in _chunk_widths(n):
        out[:, off : off + KD * cw] = xr[:, :, c0 : c0 + cw].reshape(P, KD * cw)
        off += KD * cw
        c0 += cw
    return out


def _route(xf, w_router):
    """Top-2 routing identical (up to fp rounding) to the jax reference."""
    logits = xf @ w_router.astype(np.float32)  # [T, E]
    # softmax is monotone: top-2 of probs == top-2 of logits, stable ties
    top2 = np.argsort(-logits, axis=1, kind="stable")[:, :2]  # [T, 2]
    lv = np.take_along_axis(logits, top2, axis=1)
    ev = np.exp(lv - lv[:, 0:1])
    gates = ev / ev.sum(axis=1, keepdims=True)  # [T, 2] renormalized
    return top2, gates


def kernel(x, w_router, w1, w3, w2, sw1, sw3, sw2):
    B, SEQ, D = x.shape
    T = B * SEQ
    E, _, H = w1.shape
    assert E == N_CORES
    S = T // N_CORES

    x = np.asarray(x, dtype=np.float32)
    xf = np.ascontiguousarray(x.reshape(T, D))
    top2, gates = _route(xf, np.asarray(w_router, np.float32))

    # per-expert token lists + gate values
    flat_e = top2.ravel()  # slot 2t, 2t+1 -> token t
    flat_g = gates.ravel().astype(np.float32)
    order = np.argsort(flat_e, kind="stable")
    sorted_e = flat_e[order]
    starts = np.searchsorted(sorted_e, np.arange(E + 1))
    tok_by_e = [order[starts[e] : starts[e + 1]] >> 1 for e in range(E)]
    gate_by_e = [flat_g[order[starts[e] : starts[e + 1]]] for e in range(E)]
    counts = np.diff(starts)

    # capacity: fixed floor so the compiled program is reused across calls
    C = max(1152, (int(counts.max()) + 127) // 128 * 128)

    nc = _get_program(D, H, C, S)

    w1 = np.asarray(w1, np.float32)
    w3 = np.asarray(w3, np.float32)
    w2 = np.asarray(w2, np.float32)
    sw1 = np.ascontiguousarray(np.asarray(sw1, np.float32))
    sw3 = np.ascontiguousarray(np.asarray(sw3, np.float32))
    sw2 = np.ascontiguousarray(np.asarray(sw2, np.float32))

    in_maps = []
    for e in range(E):
        n_e = int(counts[e])
        xe_pad = np.zeros((C, D), np.float32)
        xe_pad[:n_e] = xf[tok_by_e[e]]
        xeT = _pack_xT(xe_pad)
        ge = np.zeros((C, 1), np.float32)
        ge[:n_e, 0] = gate_by_e[e]
        xsT = _pack_xT(xf[e * S : (e + 1) * S])
        in_maps.append(
            {
                "xeT": xeT,
                "ge": ge,
                "xsT": xsT,
                "w1": np.ascontiguousarray(w1[e]),
                "w3": np.ascontiguousarray(w3[e]),
                "w2": np.ascontiguousarray(w2[e]),
                "sw1": sw1,
                "sw3": sw3,
                "sw2": sw2,
            }
        )

    global LAST_RESULTS
    LAST_RESULTS = run_bass_kernel_spmd(nc, in_maps, core_ids=list(range(N_CORES)))
    res = LAST_RESULTS.results

    out = np.empty((T, D), np.float32)
    for c in range(N_CORES):
        out[c * S : (c + 1) * S] = res[c]["se"]
    for e in range(E):
        n_e = int(counts[e])
        if n_e:
            out[tok_by_e[e]] += res[e]["ye"][:n_e]
    return out.reshape(B, SEQ, D)


# revision 20
# speedup vs baseline: 1.1747x; 1.0163x over previous
"""MoE feed-forward (top-2 routing + shared expert) on 8 Trainium2 cores.

Strategy (expert parallel):
  - Host computes the router (tiny [T,D]@[D,E] matmul), top-2 expert ids and
    renormalized gates, then dispatches each expert's tokens (transposed,
    capacity-padded) to the core that owns that expert's weights.
  - Core e computes  ye = (silu(xe@w1_e) * (xe@w3_e)) @ w2_e, row-scaled by the
    gate, plus a 1/8 token-slice of the always-active shared expert.
  - Host scatter-adds routed outputs into the shared-expert output.

On-device matmuls use float32r (fp32 data with fp22 multiplies, fp32
accumulation) which runs the PE at full rate for free dims >= 256.

Pipeline shape: tokens are processed in chunks of <=512 columns, widest
first. Per chunk: phase 1 (h1/h3/swiglu-gate, streaming w1/w3 tiles) then
phase 2 (down-projection against the SBUF-resident w2, staged in quarters so
phase 2 unblocks progressively). Activations ship pre-packed partition-major
so every DMA is contiguous per partition.
"""

import numpy as np

import concourse.bass as bass
import concourse.mybir as mybir
import concourse.tile as tile
from concourse import bacc
from concourse.bass_utils import run_bass_kernel_spmd

P = 128
N_CORES = 8
F32 = mybir.dt.float32
F32R = mybir.dt.float32r
AF = mybir.ActivationFunctionType

# h-tiles of w1/w3 fetched per DMA (bigger transfers, fewer descriptors)
H_BLOCK = 2


def _chunk_widths(n):
    """Split n (multiple of 128) into widths of 256..512 (multiples of 128):
    float32r matmuls run at full PE rate only for free dim >= 256.
    Widest chunk first: phase-1 weight traffic per chunk is constant, so wide
    chunks (more PE time per chunk) keep the DMA stream ahead of the PE."""
    assert n % P == 0
    if n < 2 * P:
        return [n]
    widths = []
    while n > 0:
        if n >= 640 or n == 512:
            widths.append(512)
            n -= 512
        else:  # 256, 384 (n==640 handled above)
            widths.append(n)
            n = 0
    return widths


def _swiglu_block(
    tc,
    pools,
    xT_ap,
    n_rows,
    w1_ap,
    w3_ap,
    w2_ap,
    out_ap,
    ge_ap,
    pending,
    use_silu=True,
):
    """Emit one SwiGLU y = (silu(x@w1) * (x@w3)) @ w2 over n_rows tokens.

    xT_ap: [P, (D//P)*n_rows] pre-packed activations (see _pack_xT),
    out_ap: [n_rows, D]. If ge_ap ([n_rows, 1]) is given, output rows are
    scaled by it.
    """
    nc = tc.nc
    D = w1_ap.shape[0]
    H = w1_ap.shape[1]
    KD = D // P
    KH = H // P
    ND = D // 512  # output free-dim tiles

    xpool, w2pool, wpool, gpool, spool, opool, gepool, pp1, pp3, ppo = pools

    gecell = [None]

    def _get_gate():
        if gecell[0] is None:
            gecell[0] = gepool.tile([P, n_rows // P], F32, tag="ge", name="get_")
            nc.sync.dma_start(
                gecell[0][:], ge_ap.rearrange("(c p) one -> p (c one)", p=P)
            )
        return gecell[0]

    w1r = w1_ap.rearrange("(k p) h -> p k h", p=P)
    w3r = w3_ap.rearrange("(k p) h -> p k h", p=P)
    w2r = w2_ap.rearrange("(k p) d -> p k d", p=P)
    w2cell = [None, set()]  # loaded lazily, one D-column half per dn pass

    def _stage_w2(dn):
        if w2cell[0] is None:
            w2cell[0] = w2pool.tile([P, KH, D], F32R, tag="w2res", name="w2t")
        if dn not in w2cell[1]:
            w2cell[1].add(dn)
            nc.sync.dma_start(
                w2cell[0][:, :, dn * 512 : (dn + 1) * 512],
                w2r[:, :, dn * 512 : (dn + 1) * 512],
            )

    def emit_unit(gt, c0, ct, dn):
        """Phase-2 unit: out[c0+ct*P : +P, dn*512 : +512]."""
        po = ppo.tile([P, 512], F32, tag="po", name="po")
        for kh in range(KH):
            nc.tensor.matmul(
                po,
                gt[:, kh, ct * P : (ct + 1) * P],
                w2cell[0][:, kh, dn * 512 : (dn + 1) * 512],
                start=(kh == 0),
                stop=(kh == KH - 1),
            )
        ot = opool.tile([P, 512], F32, tag="ot", name="ot")
        if ge_ap is not None:
            nc.vector.tensor_scalar_mul(
                ot[:], po, _get_gate()[:, c0 // P + ct : c0 // P + ct + 1]
            )
        else:
            nc.vector.tensor_copy(ot[:], po)
        nc.sync.dma_start(
            out_ap[c0 + ct * P : c0 + (ct + 1) * P, dn * 512 : (dn + 1) * 512],
            ot[:],
        )

    c0 = 0
    off = 0
    for cw in _chunk_widths(n_rows):
        # per-chunk activation slice (double-buffered: next chunk prefetches);
        # host packs xT chunk-major so each load is contiguous per partition
        xt = xpool.tile([P, KD, 512], F32R, tag="xT", name="xt")[:, :, :cw]
        xsrc = xT_ap[:, off : off + KD * cw].rearrange("p (k c) -> p k c", k=KD)
        kh2 = KD // 2
        nc.sync.dma_start(xt[:, :kh2, :], xsrc[:, :kh2, :])
        nc.sync.dma_start(xt[:, kh2:, :], xsrc[:, kh2:, :])
        off += KD * cw

        # ---- phase 1: gT[h, c] = silu(h1T) * h3T for this chunk ----
        gt = gpool.tile([P, KH, 512], F32R, tag="gT", name="gt")
        for hb in range(KH // H_BLOCK):
            w1t = wpool.tile([P, KD, H_BLOCK * P], F32R, tag="w1t", name="w1t")
            nc.sync.dma_start(
                w1t[:], w1r[:, :, hb * H_BLOCK * P : (hb + 1) * H_BLOCK * P]
            )
            w3t = wpool.tile([P, KD, H_BLOCK * P], F32R, tag="w3t", name="w3t")
            nc.sync.dma_start(
                w3t[:], w3r[:, :, hb * H_BLOCK * P : (hb + 1) * H_BLOCK * P]
            )
            for hi in range(H_BLOCK):
                h = hb * H_BLOCK + hi
                p1 = pp1.tile([P, 512], F32, tag="p1", name="p1")[:, :cw]
                p3 = pp3.tile([P, 512], F32, tag="p3", name="p3")[:, :cw]
                for k in range(KD):
                    nc.tensor.matmul(
                        p1,
                        w1t[:, k, hi * P : (hi + 1) * P],
                        xt[:, k, :],
                        start=(k == 0),
                        stop=(k == KD - 1),
                    )
                for k in range(KD):
                    nc.tensor.matmul(
                        p3,
                        w3t[:, k, hi * P : (hi + 1) * P],
                        xt[:, k, :],
                        start=(k == 0),
                        stop=(k == KD - 1),
                    )
                if use_silu:
                    nc.scalar.activation(gt[:, h, :cw], p1, AF.Silu)
                    nc.vector.tensor_mul(gt[:, h, :cw], gt[:, h, :cw], p3)
                else:  # silu(a) = a * sigmoid(a); CoreSim has no Silu table
                    s1 = spool.tile([P, 512], F32, tag="s1", name="s1")[:, :cw]
                    nc.scalar.activation(s1, p1, AF.Sigmoid)
                    nc.vector.tensor_mul(gt[:, h, :cw], p1, p3)
                    nc.vector.tensor_mul(gt[:, h, :cw], gt[:, h, :cw], s1)

        # ---- phase 2 for this chunk (dn-major: second w2 half loads
        # while the first half's units run) ----
        for dn in range(ND):
            _stage_w2(dn)
            for ct in range(cw // P):
                emit_unit(gt, c0, ct, dn)
        c0 += cw


def build_moe_program(D, H, C, S, use_silu=True):
    """SPMD program: routed expert over C capacity rows + shared expert over
    S token-slice rows. Same NEFF on all 8 cores, per-core input data."""
    nc = bacc.Bacc(
        "TRN2", target_bir_lowering=False, debug=False, num_devices=N_CORES
    )

    def din(name, shape, dt=F32):
        return nc.dram_tensor(name, shape, dt, kind="ExternalInput").ap()

    def dout(name, shape):
        return nc.dram_tensor(name, shape, F32, kind="ExternalOutput").ap()

    xeT = din("xeT", [P, (D // P) * C], F32R)
    ge = din("ge", [C, 1])
    xsT = din("xsT", [P, (D // P) * S], F32R)
    w1 = din("w1", [D, H], F32R)
    w3 = din("w3", [D, H], F32R)
    w2 = din("w2", [H, D], F32R)
    sw1 = din("sw1", [D, H], F32R)
    sw3 = din("sw3", [D, H], F32R)
    sw2 = din("sw2", [H, D], F32R)
    ye = dout("ye", [C, D])
    se = dout("se", [S, D])

    with tile.TileContext(nc) as tc:
        from contextlib import ExitStack

        with ExitStack() as ctx:
            pools = (
                ctx.enter_context(tc.tile_pool(name="xT", bufs=2)),
                ctx.enter_context(tc.tile_pool(name="w2res", bufs=1)),
                ctx.enter_context(tc.tile_pool(name="wstream", bufs=4)),
                ctx.enter_context(tc.tile_pool(name="gT", bufs=1)),
                ctx.enter_context(tc.tile_pool(name="stemp", bufs=2)),
                ctx.enter_context(tc.tile_pool(name="otile", bufs=3)),
                ctx.enter_context(tc.tile_pool(name="gate", bufs=1)),
                ctx.enter_context(tc.tile_pool(name="ps1", bufs=2, space="PSUM")),
                ctx.enter_context(tc.tile_pool(name="ps3", bufs=2, space="PSUM")),
                ctx.enter_context(tc.tile_pool(name="pso", bufs=2, space="PSUM")),
            )
            pending = []
            _swiglu_block(
                tc, pools, xeT, C, w1, w3, w2, ye, ge, pending, use_silu
            )
            _swiglu_block(
                tc, pools, xsT, S, sw1, sw3, sw2, se, None, pending, use_silu
            )
            for unit in pending:
                unit()

    nc.compile()
    return nc


_PROGRAM_CACHE = {}
LAST_RESULTS = None  # BassKernelResults of the most recent device run (for test.py)


def _get_program(D, H, C, S):
    key = (D, H, C, S)
    if key not in _PROGRAM_CACHE:
        _PROGRAM_CACHE[key] = build_moe_program(D, H, C, S)
    return _PROGRAM_CACHE[key]


def _pack_xT(xmat):
    """[n, D] row-major tokens -> [P, KD*n] partition-major, chunk-contiguous
    layout matching _swiglu_block's per-chunk loads."""
    n, D = xmat.shape
    KD = D // P
    xr = xmat.reshape(n, KD, P).transpose(2, 1, 0)  # [P, KD, n]
    out = np.empty((P, KD * n), np.float32)
    off = 0
    c0 = 0
    for cw in _chunk_widths(n):
        out[:, off : off + KD * cw] = xr[:, :, c0 : c0 + cw].reshape(P, KD * cw)
        off += KD * cw
        c0 += cw
    return out


def _route(xf, w_router):
    """Top-2 routing identical (up to fp rounding) to the jax reference."""
    logits = xf @ w_router.astype(np.float32)  # [T, E]
    # softmax is monotone: top-2 of probs == top-2 of logits, stable ties
    top2 = np.argsort(-logits, axis=1, kind="stable")[:, :2]  # [T, 2]
    lv = np.take_along_axis(logits, top2, axis=1)
    ev = np.exp(lv - lv[:, 0:1])
    gates = ev / ev.sum(axis=1, keepdims=True)  # [T, 2] renormalized
    return top2, gates


def kernel(x, w_router, w1, w3, w2, sw1, sw3, sw2):
    B, SEQ, D = x.shape
    T = B * SEQ
    E, _, H = w1.shape
    assert E == N_CORES
    S = T // N_CORES

    x = np.asarray(x, dtype=np.float32)
    xf = np.ascontiguousarray(x.reshape(T, D))
    top2, gates = _route(xf, np.asarray(w_router, np.float32))

    # per-expert token lists + gate values
    flat_e = top2.ravel()  # slot 2t, 2t+1 -> token t
    flat_g = gates.ravel().astype(np.float32)
    order = np.argsort(flat_e, kind="stable")
    sorted_e = flat_e[order]
    starts = np.searchsorted(sorted_e, np.arange(E + 1))
    tok_by_e = [order[starts[e] : starts[e + 1]] >> 1 for e in range(E)]
    gate_by_e = [flat_g[order[starts[e] : starts[e + 1]]] for e in range(E)]
    counts = np.diff(starts)

    # capacity: fixed floor so the compiled program is reused across calls
    C = max(1152, (int(counts.max()) + 127) // 128 * 128)

    nc = _get_program(D, H, C, S)

    w1 = np.asarray(w1, np.float32)
    w3 = np.asarray(w3, np.float32)
    w2 = np.asarray(w2, np.float32)
    sw1 = np.ascontiguousarray(np.asarray(sw1, np.float32))
    sw3 = np.ascontiguousarray(np.asarray(sw3, np.float32))
    sw2 = np.ascontiguousarray(np.asarray(sw2, np.float32))

    in_maps = []
    for e in range(E):
        n_e = int(counts[e])
        xe_pad = np.zeros((C, D), np.float32)
        xe_pad[:n_e] = xf[tok_by_e[e]]
        xeT = _pack_xT(xe_pad)
        ge = np.zeros((C, 1), np.float32)
        ge[:n_e, 0] = gate_by_e[e]
        xsT = _pack_xT(xf[e * S : (e + 1) * S])
        in_maps.append(
            {
                "xeT": xeT,
                "ge": ge,
                "xsT": xsT,
                "w1": np.ascontiguousarray(w1[e]),
                "w3": np.ascontiguousarray(w3[e]),
                "w2": np.ascontiguousarray(w2[e]),
                "sw1": sw1,
                "sw3": sw3,
                "sw2": sw2,
            }
        )

    global LAST_RESULTS
    LAST_RESULTS = run_bass_kernel_spmd(nc, in_maps, core_ids=list(range(N_CORES)))
    res = LAST_RESULTS.results

    out = np.empty((T, D), np.float32)
    for c in range(N_CORES):
        out[c * S : (c + 1) * S] = res[c]["se"]
    for e in range(E):
        n_e = int(counts[e])
        if n_e:
            out[tok_by_e[e]] += res[e]["ye"][:n_e]
    return out.reshape(B, SEQ, D)


# revision 21
# speedup vs baseline: 1.1892x; 1.0123x over previous
"""MoE feed-forward (top-2 routing + shared expert) on 8 Trainium2 cores.

Strategy (expert parallel):
  - Host computes the router (tiny [T,D]@[D,E] matmul), top-2 expert ids and
    renormalized gates, then dispatches each expert's tokens (transposed,
    capacity-padded) to the core that owns that expert's weights.
  - Core e computes  ye = (silu(xe@w1_e) * (xe@w3_e)) @ w2_e, row-scaled by the
    gate, plus a 1/8 token-slice of the always-active shared expert.
  - Host scatter-adds routed outputs into the shared-expert output.

On-device matmuls use float32r (fp32 data with fp22 multiplies, fp32
accumulation) which runs the PE at full rate for free dims >= 256.

Pipeline shape: tokens are processed in chunks of <=512 columns, widest
first. Per chunk: phase 1 (h1/h3/swiglu-gate, streaming w1/w3 tiles) then
phase 2 (down-projection against the SBUF-resident w2, staged in quarters so
phase 2 unblocks progressively). Activations ship pre-packed partition-major
so every DMA is contiguous per partition.
"""

import numpy as np

import concourse.bass as bass
import concourse.mybir as mybir
import concourse.tile as tile
from concourse import bacc
from concourse.bass_utils import run_bass_kernel_spmd

P = 128
N_CORES = 8
F32 = mybir.dt.float32
F32R = mybir.dt.float32r
AF = mybir.ActivationFunctionType

# h-tiles of w1/w3 fetched per DMA (bigger transfers, fewer descriptors)
H_BLOCK = 2


def _chunk_widths(n):
    """Split n (multiple of 128) into chunk widths from {256, 384, 512}.

    Phase-1 cost per chunk is 256 matmuls at max(LDW ~191ns, width/2.4GHz):
    the f32r weight load floors every matmul at ~191ns, so widths <= 384 are
    all equally priced and 512 costs ~213ns. A small DP picks the mix with
    minimum total (which also minimizes chunk count, i.e. w1/w3 re-reads).
    Widest first so the DMA stream stays ahead of the PE."""
    assert n % P == 0
    u = n // P
    if u <= 4:
        return [n]
    cost = {2: 191, 3: 191, 4: 213}
    dp = [None] * (u + 1)
    dp[0] = (0, 0, ())
    for i in range(1, u + 1):
        cands = []
        for w in (2, 3, 4):
            if i - w >= 0 and dp[i - w] is not None:
                c, k, ws = dp[i - w]
                cands.append((c + cost[w], k + 1, ws + (w,)))
        if cands:
            dp[i] = min(cands)
    if dp[u] is None:
        return [n]
    return sorted((w * P for w in dp[u][2]), reverse=True)


def _swiglu_block(
    tc,
    pools,
    xT_ap,
    n_rows,
    w1_ap,
    w3_ap,
    w2_ap,
    out_ap,
    ge_ap,
    pending,
    use_silu=True,
):
    """Emit one SwiGLU y = (silu(x@w1) * (x@w3)) @ w2 over n_rows tokens.

    xT_ap: [P, (D//P)*n_rows] pre-packed activations (see _pack_xT),
    out_ap: [n_rows, D]. If ge_ap ([n_rows, 1]) is given, output rows are
    scaled by it.
    """
    nc = tc.nc
    D = w1_ap.shape[0]
    H = w1_ap.shape[1]
    KD = D // P
    KH = H // P
    ND = D // 512  # output free-dim tiles

    xpool, w2pool, wpool, gpool, spool, opool, gepool, pp1, pp3, ppo = pools

    gecell = [None]

    def _get_gate():
        if gecell[0] is None:
            gecell[0] = gepool.tile([P, n_rows // P], F32, tag="ge", name="get_")
            nc.sync.dma_start(
                gecell[0][:], ge_ap.rearrange("(c p) one -> p (c one)", p=P)
            )
        return gecell[0]

    w1r = w1_ap.rearrange("(k p) h -> p k h", p=P)
    w3r = w3_ap.rearrange("(k p) h -> p k h", p=P)
    w2r = w2_ap.rearrange("(k p) d -> p k d", p=P)
    w2cell = [None, set()]  # loaded lazily, one D-column half per dn pass

    def _stage_w2(dn):
        if w2cell[0] is None:
            w2cell[0] = w2pool.tile([P, KH, D], F32R, tag="w2res", name="w2t")
        if dn not in w2cell[1]:
            w2cell[1].add(dn)
            nc.sync.dma_start(
                w2cell[0][:, :, dn * 512 : (dn + 1) * 512],
                w2r[:, :, dn * 512 : (dn + 1) * 512],
            )

    def emit_unit(gt, c0, ct, dn):
        """Phase-2 unit: out[c0+ct*P : +P, dn*512 : +512]."""
        po = ppo.tile([P, 512], F32, tag="po", name="po")
        for kh in range(KH):
            nc.tensor.matmul(
                po,
                gt[:, kh, ct * P : (ct + 1) * P],
                w2cell[0][:, kh, dn * 512 : (dn + 1) * 512],
                start=(kh == 0),
                stop=(kh == KH - 1),
            )
        ot = opool.tile([P, 512], F32, tag="ot", name="ot")
        if ge_ap is not None:
            nc.vector.tensor_scalar_mul(
                ot[:], po, _get_gate()[:, c0 // P + ct : c0 // P + ct + 1]
            )
        else:
            nc.vector.tensor_copy(ot[:], po)
        nc.sync.dma_start(
            out_ap[c0 + ct * P : c0 + (ct + 1) * P, dn * 512 : (dn + 1) * 512],
            ot[:],
        )

    c0 = 0
    off = 0
    for cw in _chunk_widths(n_rows):
        # per-chunk activation slice (double-buffered: next chunk prefetches);
        # host packs xT chunk-major so each load is contiguous per partition
        xt = xpool.tile([P, KD, 512], F32R, tag="xT", name="xt")[:, :, :cw]
        xsrc = xT_ap[:, off : off + KD * cw].rearrange("p (k c) -> p k c", k=KD)
        kstep = KD // 4 if KD % 4 == 0 else KD // 2 if KD % 2 == 0 else KD
        for k0 in range(0, KD, kstep):
            nc.sync.dma_start(
                xt[:, k0 : k0 + kstep, :], xsrc[:, k0 : k0 + kstep, :]
            )
        off += KD * cw

        # ---- phase 1: gT[h, c] = silu(h1T) * h3T for this chunk ----
        gt = gpool.tile([P, KH, 512], F32R, tag="gT", name="gt")
        for hb in range(KH // H_BLOCK):
            w1t = wpool.tile([P, KD, H_BLOCK * P], F32R, tag="w1t", name="w1t")
            nc.sync.dma_start(
                w1t[:], w1r[:, :, hb * H_BLOCK * P : (hb + 1) * H_BLOCK * P]
            )
            w3t = wpool.tile([P, KD, H_BLOCK * P], F32R, tag="w3t", name="w3t")
            nc.sync.dma_start(
                w3t[:], w3r[:, :, hb * H_BLOCK * P : (hb + 1) * H_BLOCK * P]
            )
            for hi in range(H_BLOCK):
                h = hb * H_BLOCK + hi
                p1 = pp1.tile([P, 512], F32, tag="p1", name="p1")[:, :cw]
                p3 = pp3.tile([P, 512], F32, tag="p3", name="p3")[:, :cw]
                for k in range(KD):
                    nc.tensor.matmul(
                        p1,
                        w1t[:, k, hi * P : (hi + 1) * P],
                        xt[:, k, :],
                        start=(k == 0),
                        stop=(k == KD - 1),
                    )
                for k in range(KD):
                    nc.tensor.matmul(
                        p3,
                        w3t[:, k, hi * P : (hi + 1) * P],
                        xt[:, k, :],
                        start=(k == 0),
                        stop=(k == KD - 1),
                    )
                if use_silu:
                    nc.scalar.activation(gt[:, h, :cw], p1, AF.Silu)
                    nc.vector.tensor_mul(gt[:, h, :cw], gt[:, h, :cw], p3)
                else:  # silu(a) = a * sigmoid(a); CoreSim has no Silu table
                    s1 = spool.tile([P, 512], F32, tag="s1", name="s1")[:, :cw]
                    nc.scalar.activation(s1, p1, AF.Sigmoid)
                    nc.vector.tensor_mul(gt[:, h, :cw], p1, p3)
                    nc.vector.tensor_mul(gt[:, h, :cw], gt[:, h, :cw], s1)

        # ---- phase 2 for this chunk (dn-major: second w2 half loads
        # while the first half's units run) ----
        for dn in range(ND):
            _stage_w2(dn)
            for ct in range(cw // P):
                emit_unit(gt, c0, ct, dn)
        c0 += cw


def build_moe_program(D, H, C, S, use_silu=True):
    """SPMD program: routed expert over C capacity rows + shared expert over
    S token-slice rows. Same NEFF on all 8 cores, per-core input data."""
    nc = bacc.Bacc(
        "TRN2", target_bir_lowering=False, debug=False, num_devices=N_CORES
    )

    def din(name, shape, dt=F32):
        return nc.dram_tensor(name, shape, dt, kind="ExternalInput").ap()

    def dout(name, shape):
        return nc.dram_tensor(name, shape, F32, kind="ExternalOutput").ap()

    xeT = din("xeT", [P, (D // P) * C], F32R)
    ge = din("ge", [C, 1])
    xsT = din("xsT", [P, (D // P) * S], F32R)
    w1 = din("w1", [D, H], F32R)
    w3 = din("w3", [D, H], F32R)
    w2 = din("w2", [H, D], F32R)
    sw1 = din("sw1", [D, H], F32R)
    sw3 = din("sw3", [D, H], F32R)
    sw2 = din("sw2", [H, D], F32R)
    ye = dout("ye", [C, D])
    se = dout("se", [S, D])

    with tile.TileContext(nc) as tc:
        from contextlib import ExitStack

        with ExitStack() as ctx:
            pools = (
                ctx.enter_context(tc.tile_pool(name="xT", bufs=2)),
                ctx.enter_context(tc.tile_pool(name="w2res", bufs=1)),
                ctx.enter_context(tc.tile_pool(name="wstream", bufs=4)),
                ctx.enter_context(tc.tile_pool(name="gT", bufs=1)),
                ctx.enter_context(tc.tile_pool(name="stemp", bufs=2)),
                ctx.enter_context(tc.tile_pool(name="otile", bufs=3)),
                ctx.enter_context(tc.tile_pool(name="gate", bufs=1)),
                ctx.enter_context(tc.tile_pool(name="ps1", bufs=2, space="PSUM")),
                ctx.enter_context(tc.tile_pool(name="ps3", bufs=2, space="PSUM")),
                ctx.enter_context(tc.tile_pool(name="pso", bufs=2, space="PSUM")),
            )
            pending = []
            _swiglu_block(
                tc, pools, xeT, C, w1, w3, w2, ye, ge, pending, use_silu
            )
            _swiglu_block(
                tc, pools, xsT, S, sw1, sw3, sw2, se, None, pending, use_silu
            )
            for unit in pending:
                unit()

    nc.compile()
    return nc


_PROGRAM_CACHE = {}
LAST_RESULTS = None  # BassKernelResults of the most recent device run (for test.py)


def _get_program(D, H, C, S):
    key = (D, H, C, S)
    if key not in _PROGRAM_CACHE:
        _PROGRAM_CACHE[key] = build_moe_program(D, H, C, S)
    return _PROGRAM_CACHE[key]


def _pack_xT(xmat):
    """[n, D] row-major tokens -> [P, KD*n] partition-major, chunk-contiguous
    layout matching _swiglu_block's per-chunk loads."""
    n, D = xmat.shape
    KD = D // P
    xr = xmat.reshape(n, KD, P).transpose(2, 1, 0)  # [P, KD, n]
    out = np.empty((P, KD * n), np.float32)
    off = 0
    c0 = 0
    for cw in _chunk_widths(n):
        out[:, off : off + KD * cw] = xr[:, :, c0 : c0 + cw].reshape(P, KD * cw)
        off += KD * cw
        c0 += cw
    return out


def _route(xf, w_router):
    """Top-2 routing identical (up to fp rounding) to the jax reference."""
    logits = xf @ w_router.astype(np.float32)  # [T, E]
    # softmax is monotone: top-2 of probs == top-2 of logits, stable ties
    top2 = np.argsort(-logits, axis=1, kind="stable")[:, :2]  # [T, 2]
    lv = np.take_along_axis(logits, top2, axis=1)
    ev = np.exp(lv - lv[:, 0:1])
    gates = ev / ev.sum(axis=1, keepdims=True)  # [T, 2] renormalized
    return top2, gates


def kernel(x, w_router, w1, w3, w2, sw1, sw3, sw2):
    B, SEQ, D = x.shape
    T = B * SEQ
    E, _, H = w1.shape
    assert E == N_CORES
    S = T // N_CORES

    x = np.asarray(x, dtype=np.float32)
    xf = np.ascontiguousarray(x.reshape(T, D))
    top2, gates = _route(xf, np.asarray(w_router, np.float32))

    # per-expert token lists + gate values
    flat_e = top2.ravel()  # slot 2t, 2t+1 -> token t
    flat_g = gates.ravel().astype(np.float32)
    order = np.argsort(flat_e, kind="stable")
    sorted_e = flat_e[order]
    starts = np.searchsorted(sorted_e, np.arange(E + 1))
    tok_by_e = [order[starts[e] : starts[e + 1]] >> 1 for e in range(E)]
    gate_by_e = [flat_g[order[starts[e] : starts[e + 1]]] for e in range(E)]
    counts = np.diff(starts)

    # capacity: fixed floor so the compiled program is reused across calls
    C = max(1152, (int(counts.max()) + 127) // 128 * 128)

    nc = _get_program(D, H, C, S)

    w1 = np.asarray(w1, np.float32)
    w3 = np.asarray(w3, np.float32)
    w2 = np.asarray(w2, np.float32)
    sw1 = np.ascontiguousarray(np.asarray(sw1, np.float32))
    sw3 = np.ascontiguousarray(np.asarray(sw3, np.float32))
    sw2 = np.ascontiguousarray(np.asarray(sw2, np.float32))

    in_maps = []
    for e in range(E):
        n_e = int(counts[e])
        xe_pad = np.zeros((C, D), np.float32)
        xe_pad[:n_e] = xf[tok_by_e[e]]
        xeT = _pack_xT(xe_pad)
        ge = np.zeros((C, 1), np.float32)
        ge[:n_e, 0] = gate_by_e[e]
        xsT = _pack_xT(xf[e * S : (e + 1) * S])
        in_maps.append(
            {
                "xeT": xeT,
                "ge": ge,
                "xsT": xsT,
                "w1": np.ascontiguousarray(w1[e]),
                "w3": np.ascontiguousarray(w3[e]),
                "w2": np.ascontiguousarray(w2[e]),
                "sw1": sw1,
                "sw3": sw3,
                "sw2": sw2,
            }
        )

    global LAST_RESULTS
    LAST_RESULTS = run_bass_kernel_spmd(nc, in_maps, core_ids=list(range(N_CORES)))
    res = LAST_RESULTS.results

    out = np.empty((T, D), np.float32)
    for c in range(N_CORES):
        out[c * S : (c + 1) * S] = res[c]["se"]
    for e in range(E):
        n_e = int(counts[e])
        if n_e:
            out[tok_by_e[e]] += res[e]["ye"][:n_e]
    return out.reshape(B, SEQ, D)


# revision 22
# speedup vs baseline: 1.2399x; 1.0427x over previous
"""MoE feed-forward (top-2 routing + shared expert) on 8 Trainium2 cores.

Strategy (expert parallel):
  - Host computes the router (tiny [T,D]@[D,E] matmul), top-2 expert ids and
    renormalized gates, then dispatches each expert's tokens (transposed,
    capacity-padded) to the core that owns that expert's weights.
  - Core e computes  ye = (silu(xe@w1_e) * (xe@w3_e)) @ w2_e, row-scaled by the
    gate, plus a 1/8 token-slice of the always-active shared expert.
  - Host scatter-adds routed outputs into the shared-expert output.

On-device matmuls use float32r (fp32 data with fp22 multiplies, fp32
accumulation) which runs the PE at full rate for free dims >= 256.

Pipeline shape: tokens are processed in chunks of <=512 columns, widest
first. Per chunk: phase 1 (h1/h3/swiglu-gate, streaming w1/w3 tiles) then
phase 2 (down-projection against the SBUF-resident w2, staged in quarters so
phase 2 unblocks progressively). Activations ship pre-packed partition-major
so every DMA is contiguous per partition.
"""

import numpy as np

import concourse.bass as bass
import concourse.mybir as mybir
import concourse.tile as tile
from concourse import bacc
from concourse.bass_utils import run_bass_kernel_spmd

P = 128
N_CORES = 8
F32 = mybir.dt.float32
F32R = mybir.dt.float32r
AF = mybir.ActivationFunctionType

# h-tiles of w1/w3 fetched per DMA (bigger transfers, fewer descriptors)
H_BLOCK = 2


def _chunk_widths(n):
    """Split n (multiple of 128) into chunk widths from {256, 384, 512}.

    Phase-1 cost per chunk is 256 matmuls at max(LDW ~191ns, width/2.4GHz):
    the f32r weight load floors every matmul at ~191ns, so widths <= 384 are
    all equally priced and 512 costs ~213ns. A small DP picks the mix with
    minimum total (which also minimizes chunk count, i.e. w1/w3 re-reads).
    Widest first so the DMA stream stays ahead of the PE."""
    assert n % P == 0
    u = n // P
    if u <= 4:
        return [n]
    cost = {2: 191, 3: 191, 4: 213}
    dp = [None] * (u + 1)
    dp[0] = (0, 0, ())
    for i in range(1, u + 1):
        cands = []
        for w in (2, 3, 4):
            if i - w >= 0 and dp[i - w] is not None:
                c, k, ws = dp[i - w]
                cands.append((c + cost[w], k + 1, ws + (w,)))
        if cands:
            dp[i] = min(cands)
    if dp[u] is None:
        return [n]
    return sorted((w * P for w in dp[u][2]), reverse=True)


def _swiglu_block(
    tc,
    pools,
    xT_ap,
    n_rows,
    w1_ap,
    w3_ap,
    w2_ap,
    out_ap,
    ge_ap,
    pending,
    use_silu=True,
):
    """Emit one SwiGLU y = (silu(x@w1) * (x@w3)) @ w2 over n_rows tokens.

    xT_ap: [P, (D//P)*n_rows] pre-packed activations (see _pack_xT),
    out_ap: [n_rows, D]. If ge_ap ([n_rows, 1]) is given, output rows are
    scaled by it.
    """
    nc = tc.nc
    D = out_ap.shape[1]
    H = (w1_ap.shape[1] * P) // D
    KD = D // P
    KH = H // P
    ND = D // 512  # output free-dim tiles

    xpool, w2pool, wpool, gpool, spool, opool, gepool, pp1, pp3, ppo = pools

    gecell = [None]

    def _get_gate():
        if gecell[0] is None:
            gecell[0] = gepool.tile([P, n_rows // P], F32, tag="ge", name="get_")
            nc.sync.dma_start(
                gecell[0][:], ge_ap.rearrange("(c p) one -> p (c one)", p=P)
            )
        return gecell[0]

    hbsz = KD * H_BLOCK * P  # packed cols per h-block

    def _wsrc(ap, hb):
        return ap[:, hb * hbsz : (hb + 1) * hbsz].rearrange(
            "p (k m) -> p k m", k=KD
        )
    w2cell = [None, set()]  # loaded lazily, one D-column half per dn pass

    def _stage_w2(dn):
        if w2cell[0] is None:
            w2cell[0] = w2pool.tile([P, KH, D], F32R, tag="w2res", name="w2t")
        if dn not in w2cell[1]:
            w2cell[1].add(dn)
            nc.sync.dma_start(
                w2cell[0][:, :, dn * 512 : (dn + 1) * 512],
                w2_ap[:, dn * KH * 512 : (dn + 1) * KH * 512].rearrange(
                    "p (k m) -> p k m", k=KH
                ),
            )

    def emit_unit(gt, c0, ct, dn):
        """Phase-2 unit: out[c0+ct*P : +P, dn*512 : +512]."""
        po = ppo.tile([P, 512], F32, tag="po", name="po")
        for kh in range(KH):
            nc.tensor.matmul(
                po,
                gt[:, kh, ct * P : (ct + 1) * P],
                w2cell[0][:, kh, dn * 512 : (dn + 1) * 512],
                start=(kh == 0),
                stop=(kh == KH - 1),
            )
        ot = opool.tile([P, 512], F32, tag="ot", name="ot")
        if ge_ap is not None:
            nc.vector.tensor_scalar_mul(
                ot[:], po, _get_gate()[:, c0 // P + ct : c0 // P + ct + 1]
            )
        else:
            nc.vector.tensor_copy(ot[:], po)
        nc.sync.dma_start(
            out_ap[c0 + ct * P : c0 + (ct + 1) * P, dn * 512 : (dn + 1) * 512],
            ot[:],
        )

    c0 = 0
    off = 0
    for cw in _chunk_widths(n_rows):
        # per-chunk activation slice (double-buffered: next chunk prefetches);
        # host packs xT chunk-major so each load is contiguous per partition
        xt = xpool.tile([P, KD, 512], F32R, tag="xT", name="xt")[:, :, :cw]
        xsrc = xT_ap[:, off : off + KD * cw].rearrange("p (k c) -> p k c", k=KD)
        kstep = KD // 4 if KD % 4 == 0 else KD // 2 if KD % 2 == 0 else KD
        for k0 in range(0, KD, kstep):
            nc.sync.dma_start(
                xt[:, k0 : k0 + kstep, :], xsrc[:, k0 : k0 + kstep, :]
            )
        off += KD * cw

        # ---- phase 1: gT[h, c] = silu(h1T) * h3T for this chunk ----
        gt = gpool.tile([P, KH, 512], F32R, tag="gT", name="gt")
        for hb in range(KH // H_BLOCK):
            w1t = wpool.tile([P, KD, H_BLOCK * P], F32R, tag="w1t", name="w1t")
            nc.sync.dma_start(w1t[:], _wsrc(w1_ap, hb))
            w3t = wpool.tile([P, KD, H_BLOCK * P], F32R, tag="w3t", name="w3t")
            nc.sync.dma_start(w3t[:], _wsrc(w3_ap, hb))
            for hi in range(H_BLOCK):
                h = hb * H_BLOCK + hi
                p1 = pp1.tile([P, 512], F32, tag="p1", name="p1")[:, :cw]
                p3 = pp3.tile([P, 512], F32, tag="p3", name="p3")[:, :cw]
                for k in range(KD):
                    nc.tensor.matmul(
                        p1,
                        w1t[:, k, hi * P : (hi + 1) * P],
                        xt[:, k, :],
                        start=(k == 0),
                        stop=(k == KD - 1),
                    )
                for k in range(KD):
                    nc.tensor.matmul(
                        p3,
                        w3t[:, k, hi * P : (hi + 1) * P],
                        xt[:, k, :],
                        start=(k == 0),
                        stop=(k == KD - 1),
                    )
                if use_silu:
                    nc.scalar.activation(gt[:, h, :cw], p1, AF.Silu)
                    nc.vector.tensor_mul(gt[:, h, :cw], gt[:, h, :cw], p3)
                else:  # silu(a) = a * sigmoid(a); CoreSim has no Silu table
                    s1 = spool.tile([P, 512], F32, tag="s1", name="s1")[:, :cw]
                    nc.scalar.activation(s1, p1, AF.Sigmoid)
                    nc.vector.tensor_mul(gt[:, h, :cw], p1, p3)
                    nc.vector.tensor_mul(gt[:, h, :cw], gt[:, h, :cw], s1)

        # ---- phase 2 for this chunk (dn-major: second w2 half loads
        # while the first half's units run) ----
        for dn in range(ND):
            _stage_w2(dn)
            for ct in range(cw // P):
                emit_unit(gt, c0, ct, dn)
        c0 += cw


def build_moe_program(D, H, C, S, use_silu=True):
    """SPMD program: routed expert over C capacity rows + shared expert over
    S token-slice rows. Same NEFF on all 8 cores, per-core input data."""
    nc = bacc.Bacc(
        "TRN2", target_bir_lowering=False, debug=False, num_devices=N_CORES
    )

    def din(name, shape, dt=F32):
        return nc.dram_tensor(name, shape, dt, kind="ExternalInput").ap()

    def dout(name, shape):
        return nc.dram_tensor(name, shape, F32, kind="ExternalOutput").ap()

    xeT = din("xeT", [P, (D // P) * C], F32R)
    ge = din("ge", [C, 1])
    xsT = din("xsT", [P, (D // P) * S], F32R)
    w1 = din("w1", [P, (D // P) * H], F32R)
    w3 = din("w3", [P, (D // P) * H], F32R)
    w2 = din("w2", [P, H * D // P], F32R)
    sw1 = din("sw1", [P, (D // P) * H], F32R)
    sw3 = din("sw3", [P, (D // P) * H], F32R)
    sw2 = din("sw2", [P, H * D // P], F32R)
    ye = dout("ye", [C, D])
    se = dout("se", [S, D])

    with tile.TileContext(nc) as tc:
        from contextlib import ExitStack

        with ExitStack() as ctx:
            pools = (
                ctx.enter_context(tc.tile_pool(name="xT", bufs=2)),
                ctx.enter_context(tc.tile_pool(name="w2res", bufs=1)),
                ctx.enter_context(tc.tile_pool(name="wstream", bufs=4)),
                ctx.enter_context(tc.tile_pool(name="gT", bufs=1)),
                ctx.enter_context(tc.tile_pool(name="stemp", bufs=2)),
                ctx.enter_context(tc.tile_pool(name="otile", bufs=3)),
                ctx.enter_context(tc.tile_pool(name="gate", bufs=1)),
                ctx.enter_context(tc.tile_pool(name="ps1", bufs=2, space="PSUM")),
                ctx.enter_context(tc.tile_pool(name="ps3", bufs=2, space="PSUM")),
                ctx.enter_context(tc.tile_pool(name="pso", bufs=2, space="PSUM")),
            )
            pending = []
            _swiglu_block(
                tc, pools, xeT, C, w1, w3, w2, ye, ge, pending, use_silu
            )
            _swiglu_block(
                tc, pools, xsT, S, sw1, sw3, sw2, se, None, pending, use_silu
            )
            for unit in pending:
                unit()

    nc.compile()
    return nc


_PROGRAM_CACHE = {}
LAST_RESULTS = None  # BassKernelResults of the most recent device run (for test.py)


def _get_program(D, H, C, S):
    key = (D, H, C, S)
    if key not in _PROGRAM_CACHE:
        _PROGRAM_CACHE[key] = build_moe_program(D, H, C, S)
    return _PROGRAM_CACHE[key]


def _pack_xT(xmat):
    """[n, D] row-major tokens -> [P, KD*n] partition-major, chunk-contiguous
    layout matching _swiglu_block's per-chunk loads."""
    n, D = xmat.shape
    KD = D // P
    xr = xmat.reshape(n, KD, P).transpose(2, 1, 0)  # [P, KD, n]
    out = np.empty((P, KD * n), np.float32)
    off = 0
    c0 = 0
    for cw in _chunk_widths(n):
        out[:, off : off + KD * cw] = xr[:, :, c0 : c0 + cw].reshape(P, KD * cw)
        off += KD * cw
        c0 += cw
    return out


def _pack_w13(w):
    """[D, H] -> [P, (D//P)*H] h-block-major: each h-block's weights are one
    contiguous run per partition."""
    Dw, Hw = w.shape
    KD = Dw // P
    nhb = Hw // (H_BLOCK * P)
    return np.ascontiguousarray(
        w.reshape(KD, P, nhb, H_BLOCK * P)
        .transpose(1, 2, 0, 3)
        .reshape(P, KD * Hw)
    )


def _pack_w2(w):
    """[H, D] -> [P, H*D//P] dn-major: each 512-wide D-column half is one
    contiguous run per partition."""
    Hw, Dw = w.shape
    KH = Hw // P
    ND = Dw // 512
    return np.ascontiguousarray(
        w.reshape(KH, P, ND, 512).transpose(1, 2, 0, 3).reshape(P, Hw * Dw // P)
    )


def _route(xf, w_router):
    """Top-2 routing identical (up to fp rounding) to the jax reference."""
    logits = xf @ w_router.astype(np.float32)  # [T, E]
    # softmax is monotone: top-2 of probs == top-2 of logits, stable ties
    top2 = np.argsort(-logits, axis=1, kind="stable")[:, :2]  # [T, 2]
    lv = np.take_along_axis(logits, top2, axis=1)
    ev = np.exp(lv - lv[:, 0:1])
    gates = ev / ev.sum(axis=1, keepdims=True)  # [T, 2] renormalized
    return top2, gates


def kernel(x, w_router, w1, w3, w2, sw1, sw3, sw2):
    B, SEQ, D = x.shape
    T = B * SEQ
    E, _, H = w1.shape
    assert E == N_CORES
    S = T // N_CORES

    x = np.asarray(x, dtype=np.float32)
    xf = np.ascontiguousarray(x.reshape(T, D))
    top2, gates = _route(xf, np.asarray(w_router, np.float32))

    # per-expert token lists + gate values
    flat_e = top2.ravel()  # slot 2t, 2t+1 -> token t
    flat_g = gates.ravel().astype(np.float32)
    order = np.argsort(flat_e, kind="stable")
    sorted_e = flat_e[order]
    starts = np.searchsorted(sorted_e, np.arange(E + 1))
    tok_by_e = [order[starts[e] : starts[e + 1]] >> 1 for e in range(E)]
    gate_by_e = [flat_g[order[starts[e] : starts[e + 1]]] for e in range(E)]
    counts = np.diff(starts)

    # capacity: fixed floor so the compiled program is reused across calls
    C = max(1152, (int(counts.max()) + 127) // 128 * 128)

    nc = _get_program(D, H, C, S)

    w1 = np.asarray(w1, np.float32)
    w3 = np.asarray(w3, np.float32)
    w2 = np.asarray(w2, np.float32)
    sw1p = _pack_w13(np.asarray(sw1, np.float32))
    sw3p = _pack_w13(np.asarray(sw3, np.float32))
    sw2p = _pack_w2(np.asarray(sw2, np.float32))

    in_maps = []
    for e in range(E):
        n_e = int(counts[e])
        xe_pad = np.zeros((C, D), np.float32)
        xe_pad[:n_e] = xf[tok_by_e[e]]
        xeT = _pack_xT(xe_pad)
        ge = np.zeros((C, 1), np.float32)
        ge[:n_e, 0] = gate_by_e[e]
        xsT = _pack_xT(xf[e * S : (e + 1) * S])
        in_maps.append(
            {
                "xeT": xeT,
                "ge": ge,
                "xsT": xsT,
                "w1": _pack_w13(w1[e]),
                "w3": _pack_w13(w3[e]),
                "w2": _pack_w2(w2[e]),
                "sw1": sw1p,
                "sw3": sw3p,
                "sw2": sw2p,
            }
        )

    global LAST_RESULTS
    LAST_RESULTS = run_bass_kernel_spmd(nc, in_maps, core_ids=list(range(N_CORES)))
    res = LAST_RESULTS.results

    out = np.empty((T, D), np.float32)
    for c in range(N_CORES):
        out[c * S : (c + 1) * S] = res[c]["se"]
    for e in range(E):
        n_e = int(counts[e])
        if n_e:
            out[tok_by_e[e]] += res[e]["ye"][:n_e]
    return out.reshape(B, SEQ, D)
